# revision 1
# baseline (speedup 1.0000x reference)
"""Trainium2 Bass kernel for nn_ExpandedResolventFMNet.

Mathematical reformulation (validated in fp64 against the jax reference):

The reference builds kron(A.T, My) [8192x4096], its Gram [4096^2], resolvent
kron masks, and solves a dense 4096x4096 system.  All of that collapses:

  first        = kron(A A^T, G),              G = My^T My
  second       = kron-sum of 64x64 factors; with X = Mx W the full system is
  M(W)         = S~ W G + LMBDA * sum_d Dd*( (Dd*W) G ) = R~    (* = Hadamard)
  S~           = Mx^T (A A^T) Mx
  R~           = Mx^T A Bc^T My
  Dd           = resolvent-mask difference matrices (64x64)
  output C     = (Mx W)^T

The 4096x4096 operator kron(S~,G)+LMBDA*blockdiag is SPD with cond ~3e2; PCG
with the exact-kron preconditioner P^-1 = kron(S~^-1, G^-1) (applied as two
64x64 matmuls) converges to the fp32 floor in <=14 iterations.  The device
runs the transposed system in Y = W^T:

  M'(Y) = G Y S~ + sum_d DdT * (G (DdT * Y)),   C = Y Mx^T

and exploits symmetry so that every matmul is transpose-free:
  (G p)^T   = mm(lhsT=p,  rhs=G)     [G symmetric]
  (G p) S~  = mm(lhsT=(G p)^T, rhs=S~)
  (Gi r)^T  = mm(lhsT=r,  rhs=Gi)
  (Gi r) Si = mm(lhsT=(Gi r)^T, rhs=Si)

S~^-1 and G^-1 are produced on-device by Newton-Schulz iteration.
sqrt(LMBDA) is folded into DdT.  Work is sharded over 8 cores for the
V=5000 projections (AllReduce of the 64KB partials); the small solve runs
redundantly on every core.
"""

import numpy as np

import concourse.bacc as bacc
import concourse.mybir as mybir
from concourse.bass_isa import ReduceOp
from concourse.bass_utils import run_bass_kernel_spmd
from concourse.masks import make_identity
from concourse.tile import TileContext

F32 = mybir.dt.float32
K = 64          # spectral basis size
C = 128         # feature channels
V = 5000        # vertices
CHUNK = 125     # v-contraction tile (partition dim)
N_CORES = 8
N_ITERS = 12
NEWTON_STEPS_S = 8
NEWTON_STEPS_G = 4
SQRT_LMBDA = 10.0

SHARD = True    # shard projections over cores + AllReduce partials

_PROGRAM_CACHE = {}


def build_program(shard: bool):
    nc = bacc.Bacc("TRN2", num_devices=N_CORES)
    v_local = V // N_CORES if shard else V          # 625 or 5000
    n_chunks = v_local // CHUNK                     # 5 or 40

    fx_d = nc.dram_tensor("fx", [v_local, C], F32, kind="ExternalInput")
    fy_d = nc.dram_tensor("fy", [v_local, C], F32, kind="ExternalInput")
    pxT_d = nc.dram_tensor("pxT", [v_local, K], F32, kind="ExternalInput")
    pyT_d = nc.dram_tensor("pyT", [v_local, K], F32, kind="ExternalInput")
    mx_d = nc.dram_tensor("mx", [K, K], F32, kind="ExternalInput")
    my_d = nc.dram_tensor("my", [K, K], F32, kind="ExternalInput")
    mxT_d = nc.dram_tensor("mxT", [K, K], F32, kind="ExternalInput")
    myT_d = nc.dram_tensor("myT", [K, K], F32, kind="ExternalInput")
    ev_d = nc.dram_tensor("ev", [1, 2 * K], F32, kind="ExternalInput")
    out_d = nc.dram_tensor("out", [K, K], F32, kind="ExternalOutput")

    if shard:
        ccx_in = nc.dram_tensor("ccx_in", [C, K], F32)
        ccx_out = nc.dram_tensor("ccx_out", [C, K], F32, addr_space="Shared")
        ccy_in = nc.dram_tensor("ccy_in", [C, K], F32)
        ccy_out = nc.dram_tensor("ccy_out", [C, K], F32, addr_space="Shared")

    with TileContext(nc) as tc:
        with (
            tc.tile_pool(name="big", bufs=1) as bp,
            tc.tile_pool(name="persist", bufs=1) as sp,
            tc.tile_pool(name="work", bufs=2) as wp,
            tc.tile_pool(name="psum", bufs=2, space="PSUM") as pp,
        ):

            # rotating psum tags: 3 tags x bufs=2 -> 6 banks (+proj acc 2 = 8)
            _ps_state = {"i": 0}

            def ps_tile(shape):
                i = _ps_state["i"]
                _ps_state["i"] += 1
                return pp.tile(shape, F32, tag=f"ps{i % 3}", name=f"pst{i}")

            def sb_copy(src_psum, shape, pool, tag, engine="vector"):
                t = pool.tile(shape, F32, tag=tag, name=tag)
                if engine == "vector":
                    nc.vector.tensor_copy(t, src_psum)
                else:
                    nc.scalar.copy(t, src_psum)
                return t

            # ---------------- input DMA (one DMA per big tensor) ------------
            fx_t = bp.tile([CHUNK, n_chunks, C], F32)
            fy_t = bp.tile([CHUNK, n_chunks, C], F32)
            pxT_t = bp.tile([CHUNK, n_chunks, K], F32)
            pyT_t = bp.tile([CHUNK, n_chunks, K], F32)
            nc.sync.dma_start(
                fx_t, fx_d.rearrange("(n p) c -> p n c", p=CHUNK))
            nc.sync.dma_start(
                fy_t, fy_d.rearrange("(n p) c -> p n c", p=CHUNK))
            nc.sync.dma_start(
                pxT_t, pxT_d.rearrange("(n p) c -> p n c", p=CHUNK))
            nc.sync.dma_start(
                pyT_t, pyT_d.rearrange("(n p) c -> p n c", p=CHUNK))
            mx_s = sp.tile([K, K], F32)
            my_s = sp.tile([K, K], F32)
            mxT_s = sp.tile([K, K], F32)
            myT_s = sp.tile([K, K], F32)
            ev_t = sp.tile([1, 2 * K], F32)
            nc.sync.dma_start(mx_s, mx_d[:, :])
            nc.sync.dma_start(my_s, my_d[:, :])
            nc.sync.dma_start(mxT_s, mxT_d[:, :])
            nc.sync.dma_start(myT_s, myT_d[:, :])
            nc.sync.dma_start(ev_t, ev_d[:, :])

            ident = sp.tile([C, C], F32)
            make_identity(nc, ident)
            id64 = ident[0:K, 0:K]
            ones_row = sp.tile([1, K], F32)
            nc.vector.memset(ones_row, 1.0)
            ones_col = sp.tile([K, 1], F32)
            nc.vector.memset(ones_col, 1.0)


            # ---------------- projections: AT = fx^T pxT, ByT = fy^T pyT ----
            with tc.tile_pool(name="pacc", bufs=1, space="PSUM") as pacc:
                at_p = pacc.tile([C, K], F32)    # A^T partial  [C,K]
                byt_p = pacc.tile([C, K], F32)   # By^T partial [C,K]
                for n in range(n_chunks):
                    nc.tensor.matmul(at_p, fx_t[:, n, :], pxT_t[:, n, :],
                                     start=(n == 0), stop=(n == n_chunks - 1))
                if shard:
                    # x-side collective issues while the y-side projections run
                    partx_s = sp.tile([C, K], F32)
                    nc.vector.tensor_copy(partx_s, at_p)
                    nc.sync.dma_start(ccx_in[:, :], partx_s)
                    nc.gpsimd.collective_compute(
                        "AllReduce", mybir.AluOpType.add,
                        replica_groups=[list(range(N_CORES))],
                        ins=[ccx_in[:, :]], outs=[ccx_out[:, :]])
                for n in range(n_chunks):
                    nc.tensor.matmul(byt_p, fy_t[:, n, :], pyT_t[:, n, :],
                                     start=(n == 0), stop=(n == n_chunks - 1))
                if shard:
                    party_s = sp.tile([C, K], F32)
                    nc.vector.tensor_copy(party_s, byt_p)
                    nc.sync.dma_start(ccy_in[:, :], party_s)
                    nc.gpsimd.collective_compute(
                        "AllReduce", mybir.AluOpType.add,
                        replica_groups=[list(range(N_CORES))],
                        ins=[ccy_in[:, :]], outs=[ccy_out[:, :]])
                else:
                    at_s = sb_copy(at_p, [C, K], sp, "at_s")
                    byt_s = sb_copy(byt_p, [C, K], sp, "byt_s")

            # ------- collective-independent work first (hides CC latency) ---
            # G = My^T My
            g_p = ps_tile([K, K])
            nc.tensor.matmul(g_p, my_s, my_s)
            g_s = sb_copy(g_p, [K, K], sp, "g_s")

            # resolvent masks: ev = [ex | ey]; t = ev/max(ev); im = 1/(1+t);
            # re = sqrt(t)*im; both scaled by sqrt(LMBDA)
            evmax = sp.tile([1, 1], F32)
            nc.vector.tensor_reduce(evmax, ev_t, mybir.AxisListType.X,
                                    mybir.AluOpType.max)
            evrec = sp.tile([1, 1], F32)
            nc.vector.reciprocal(evrec, evmax)
            t_t = sp.tile([1, 2 * K], F32)
            nc.vector.tensor_scalar_mul(t_t, ev_t, evrec)
            tp1 = sp.tile([1, 2 * K], F32)
            nc.vector.tensor_scalar_add(tp1, t_t, 1.0)
            im_t = sp.tile([1, 2 * K], F32)
            nc.vector.reciprocal(im_t, tp1)
            sq_t = sp.tile([1, 2 * K], F32)
            nc.scalar.sqrt(sq_t, t_t)
            re_t = sp.tile([1, 2 * K], F32)
            nc.vector.tensor_mul(re_t, sq_t, im_t)
            nc.vector.tensor_scalar_mul(re_t, re_t, SQRT_LMBDA)
            nc.vector.tensor_scalar_mul(im_t, im_t, SQRT_LMBDA)

            # D1T[a,i] = re2[a] - re1[i]; D2T likewise from im
            d_s = []
            for idx, src in enumerate((re_t, im_t)):
                pa = ps_tile([K, K])
                nc.tensor.matmul(pa, src[0:1, K:2 * K], ones_row)  # v2[p]
                pb = ps_tile([K, K])
                nc.tensor.matmul(pb, ones_row, src[0:1, 0:K])      # v1[f]
                ta = sb_copy(pa, [K, K], sp, f"dta{idx}")
                dt = sp.tile([K, K], F32, tag=f"d{idx}t_s", name=f"d{idx}t_s")
                nc.vector.tensor_sub(dt, ta, pb)
                d_s.append(dt)
            d1t_s, d2t_s = d_s
            d12t_s = sp.tile([K, 2 * K], F32)
            nc.vector.tensor_copy(d12t_s[:, 0:K], d1t_s)
            nc.vector.tensor_copy(d12t_s[:, K:2 * K], d2t_s)

            # Newton-Schulz inverse (S symmetric PD): X' = 2X - X S X
            def newton_inverse(mat_s, tag, steps):
                rs = sp.tile([K, 1], F32, tag=f"{tag}_rs", name=f"{tag}_rs")
                nc.vector.tensor_reduce(rs, mat_s, mybir.AxisListType.X,
                                        mybir.AluOpType.add,
                                        apply_absolute_value=True)
                nc.gpsimd.partition_all_reduce(rs, rs, K, ReduceOp.max)
                al = sp.tile([K, 1], F32, tag=f"{tag}_al", name=f"{tag}_al")
                nc.vector.reciprocal(al, rs)
                x_s = sp.tile([K, K], F32, tag=f"{tag}_x0", name=f"{tag}_x0")
                nc.vector.tensor_scalar_mul(x_s, id64, al)
                for it in range(steps):
                    t1 = ps_tile([K, K])
                    nc.tensor.matmul(t1, mat_s, x_s)          # S X (S sym)
                    t1s = wp.tile([K, K], F32, tag=f"{tag}_t1s",
                                  name=f"{tag}_t1s")
                    nc.vector.tensor_copy(t1s, t1)
                    t2 = ps_tile([K, K])
                    nc.tensor.matmul(t2, x_s, t1s)            # X (S X) (X sym)
                    xn = sp.tile([K, K], F32, tag=f"{tag}_x{it + 1}",
                                 name=f"{tag}_x{it + 1}")
                    nc.vector.scalar_tensor_tensor(
                        xn, x_s, 2.0, t2,
                        op0=mybir.AluOpType.mult,
                        op1=mybir.AluOpType.subtract)
                    x_s = xn
                return x_s

            gi_s = newton_inverse(g_s, "gi", NEWTON_STEPS_G)

            # ------- collective-dependent chain -----------------------------
            if shard:
                at_s = sp.tile([C, K], F32, tag="at_s", name="at_s")
                nc.sync.dma_start(at_s, ccx_out[:, :])

            # S~ = Mx^T (A A^T) Mx    [S_A symmetric -> no transpose]
            sa_p = ps_tile([K, K])
            nc.tensor.matmul(sa_p, at_s, at_s)          # A A^T
            sa_s = sb_copy(sa_p, [K, K], sp, "sa_s")
            h1t_p = ps_tile([K, K])
            nc.tensor.matmul(h1t_p, sa_s, mx_s)         # S_A Mx (sym trick)
            h1t_s = sb_copy(h1t_p, [K, K], sp, "h1t_s")
            st_p = ps_tile([K, K])
            nc.tensor.matmul(st_p, mx_s, h1t_s)         # Mx^T S_A Mx
            st_s = sb_copy(st_p, [K, K], sp, "st_s")

            si_s = newton_inverse(st_s, "si", NEWTON_STEPS_S)

            if shard:
                byt_s = sp.tile([C, K], F32, tag="byt_s", name="byt_s")
                nc.sync.dma_start(byt_s, ccy_out[:, :])

            # RHS' = My^T Bc A^T Mx = My^T (My (By A^T)) Mx
            byat_p = ps_tile([K, K])
            nc.tensor.matmul(byat_p, byt_s, at_s)       # By A^T
            byat_s = sb_copy(byat_p, [K, K], sp, "byat_s")
            bca_p = ps_tile([K, K])
            nc.tensor.matmul(bca_p, myT_s, byat_s)      # My (By A^T) = Bc A^T
            bca_s = sb_copy(bca_p, [K, K], sp, "bca_s")
            w_p = ps_tile([K, K])
            nc.tensor.matmul(w_p, my_s, bca_s)          # My^T Bc A^T
            w_s = sb_copy(w_p, [K, K], sp, "w_s")
            wt_p = ps_tile([K, K])
            nc.tensor.transpose(wt_p, w_s, id64)
            wt_s = sb_copy(wt_p, [K, K], sp, "wt_s")
            rp_p = ps_tile([K, K])
            nc.tensor.matmul(rp_p, wt_s, mx_s)          # (My^T Bc A^T) Mx
            r_s = sp.tile([K, K], F32)                  # CG residual
            nc.vector.tensor_copy(r_s, rp_p)

            # ------- PCG: pipelined (vector recurrences, exact dots) --------
            # state: y, r, z=P^-1 r, p, q=Mp, s=P^-1 q; per iteration the
            # matvec w=Mz and precond v=P^-1 w run concurrently with the
            # dot/axpy chain; p,q,s advance by the beta-recurrence.
            y_s = sp.tile([K, K], F32)
            nc.vector.memset(y_s, 0.0)
            p_s = sp.tile([K, K], F32)
            q_s = sp.tile([K, K], F32)
            s_s = sp.tile([K, K], F32)
            z_s = sp.tile([K, K], F32)
            u_s = sp.tile([K, 2 * K], F32)   # stacked [D1T*z | D2T*z]

            def precond_psum(x_tile, tag):
                """P^-1 x in PSUM via (Gi x)^T = mm(lhsT=x, rhs=Gi)."""
                ut_p = ps_tile([K, K])
                nc.tensor.matmul(ut_p, x_tile, gi_s)
                ut_s = wp.tile([K, K], F32, tag=f"{tag}_uts", name=f"{tag}_uts")
                nc.scalar.copy(ut_s, ut_p)
                v_p = ps_tile([K, K])
                nc.tensor.matmul(v_p, ut_s, si_s)
                return v_p

            def matvec_z(tag):
                """w = M z into SBUF (reads z_s)."""
                nc.vector.tensor_mul(u_s[:, 0:K], d1t_s, z_s)
                nc.vector.tensor_mul(u_s[:, K:2 * K], d2t_s, z_s)
                gzt_p = ps_tile([K, K])
                nc.tensor.matmul(gzt_p, z_s, g_s)         # (G z)^T
                gzt_s = wp.tile([K, K], F32, tag="mv_gzts", name="mv_gzts")
                nc.scalar.copy(gzt_s, gzt_p)
                t2_p = ps_tile([K, K])
                nc.tensor.matmul(t2_p, gzt_s, st_s)       # (G z) S~
                gu_p = ps_tile([K, 2 * K])
                nc.tensor.matmul(gu_p[:, 0:K], g_s, u_s[:, 0:K])   # G u1
                nc.tensor.matmul(gu_p[:, K:2 * K], g_s, u_s[:, K:2 * K])
                mm_s = wp.tile([K, 2 * K], F32, tag="mv_mm", name="mv_mm")
                nc.vector.tensor_mul(mm_s, d12t_s, gu_p)  # masked, both halves
                a1_s = wp.tile([K, K], F32, tag="mv_a1", name="mv_a1")
                nc.vector.tensor_add(a1_s, mm_s[:, 0:K], t2_p)
                w_s = wp.tile([K, K], F32, tag="mv_w", name="mv_w")
                nc.vector.tensor_add(w_s, a1_s, mm_s[:, K:2 * K])
                return w_s

            def dot_b(a_ap, b_ap, tag):
                """<a,b> broadcast to all partitions as [K,1] SBUF."""
                prod = wp.tile([K, K], F32, tag="dot_dm", name="dot_dm")
                acc = wp.tile([K, 1], F32, tag=f"{tag}_acc", name=f"{tag}_acc")
                nc.vector.scalar_tensor_tensor(
                    prod, a_ap, 1.0, b_ap,
                    op0=mybir.AluOpType.bypass, op1=mybir.AluOpType.mult,
                    accum_out=acc)
                nc.gpsimd.partition_all_reduce(acc, acc, K, ReduceOp.add)
                return acc

            # init: z = P^-1 r; w = Mz; v = P^-1 w; p=z, q=w, s=v
            z0_p = precond_psum(r_s, "pcz")
            nc.vector.tensor_copy(z_s, z0_p)
            nc.vector.tensor_copy(p_s, z0_p)
            rz0 = dot_b(r_s, z_s, "rz")
            rzrec = wp.tile([K, 1], F32, tag="rzrec", name="rzrec")
            nc.vector.reciprocal(rzrec, rz0)
            rzneg = wp.tile([K, 1], F32, tag="rzneg", name="rzneg")
            nc.vector.tensor_scalar_mul(rzneg, rz0, -1.0)
            w_s = matvec_z("init")
            nc.vector.tensor_copy(q_s, w_s)
            v_p = precond_psum(w_s, "pcv")
            nc.vector.tensor_copy(s_s, v_p)

            for it in range(N_ITERS):
                # ---- alpha = rz/<p,q>; r,z,y updates ----
                pq = dot_b(p_s, q_s, "pq")
                pqr = wp.tile([K, 1], F32, tag="pqr", name="pqr")
                nc.vector.reciprocal(pqr, pq)
                if it < N_ITERS - 1:
                    an = wp.tile([K, 1], F32, tag="an", name="an")
                    nc.vector.tensor_mul(an, rzneg, pqr)
                    nc.vector.scalar_tensor_tensor(
                        r_s, q_s, an, r_s,
                        op0=mybir.AluOpType.mult, op1=mybir.AluOpType.add)
                    nc.vector.scalar_tensor_tensor(
                        z_s, s_s, an, z_s,
                        op0=mybir.AluOpType.mult, op1=mybir.AluOpType.add)
                al = wp.tile([K, 1], F32, tag="al", name="al")
                nc.vector.tensor_mul(al, rz0, pqr)
                nc.vector.scalar_tensor_tensor(
                    y_s, p_s, al, y_s,
                    op0=mybir.AluOpType.mult, op1=mybir.AluOpType.add)

                if it == N_ITERS - 1:
                    break

                # ---- rz_new, beta; w/v for the NEXT q,s updates ----
                rz_new = dot_b(r_s, z_s, "rz")
                w_s = matvec_z(f"i{it}")
                if it < N_ITERS - 2:
                    v_p = precond_psum(w_s, f"pcv")
                bt = wp.tile([K, 1], F32, tag="bt", name="bt")
                nc.vector.tensor_mul(bt, rz_new, rzrec)
                nc.vector.scalar_tensor_tensor(
                    p_s, p_s, bt, z_s,
                    op0=mybir.AluOpType.mult, op1=mybir.AluOpType.add)
                nc.vector.scalar_tensor_tensor(
                    q_s, q_s, bt, w_s,
                    op0=mybir.AluOpType.mult, op1=mybir.AluOpType.add)
                if it < N_ITERS - 2:
                    nc.vector.scalar_tensor_tensor(
                        s_s, s_s, bt, v_p,
                        op0=mybir.AluOpType.mult, op1=mybir.AluOpType.add)
                rz0 = rz_new
                rzrec = wp.tile([K, 1], F32, tag="rzrec", name="rzrec")
                nc.vector.reciprocal(rzrec, rz0)
                rzneg = wp.tile([K, 1], F32, tag="rzneg", name="rzneg")
                nc.vector.tensor_scalar_mul(rzneg, rz0, -1.0)

            # ---------------- output: C = Y Mx^T ----------------
            yt_p = ps_tile([K, K])
            nc.tensor.transpose(yt_p, y_s, id64)
            yt_s = wp.tile([K, K], F32, tag="yt_s", name="yt_s")
            nc.vector.tensor_copy(yt_s, yt_p)
            c_p = ps_tile([K, K])
            nc.tensor.matmul(c_p, yt_s, mxT_s)      # Y Mx^T
            c_s = wp.tile([K, K], F32, tag="c_s", name="c_s")
            nc.vector.tensor_copy(c_s, c_p)
            nc.sync.dma_start(out_d[:, :], c_s)

    nc.finalize()
    return nc


def get_program(shard: bool):
    if shard not in _PROGRAM_CACHE:
        _PROGRAM_CACHE[shard] = build_program(shard)
    return _PROGRAM_CACHE[shard]


def make_in_maps(inputs, shard: bool):
    fx = np.ascontiguousarray(np.asarray(inputs["feat_x"], np.float32)[0])
    fy = np.ascontiguousarray(np.asarray(inputs["feat_y"], np.float32)[0])
    pxT = np.ascontiguousarray(np.asarray(inputs["evecs_trans_x"], np.float32)[0].T)
    pyT = np.ascontiguousarray(np.asarray(inputs["evecs_trans_y"], np.float32)[0].T)
    mx = np.ascontiguousarray(np.asarray(inputs["sqrtMk_x"], np.float32)[0])
    my = np.ascontiguousarray(np.asarray(inputs["sqrtMk_y"], np.float32)[0])
    ev = np.ascontiguousarray(np.concatenate([
        np.asarray(inputs["evals_x"], np.float32)[0],
        np.asarray(inputs["evals_y"], np.float32)[0],
    ])[None, :])
    small = {
        "mx": mx, "my": my,
        "mxT": np.ascontiguousarray(mx.T),
        "myT": np.ascontiguousarray(my.T),
        "ev": ev,
    }
    in_maps = []
    for c in range(N_CORES):
        if shard:
            lo, hi = c * (V // N_CORES), (c + 1) * (V // N_CORES)
            m = {"fx": fx[lo:hi], "fy": fy[lo:hi],
                 "pxT": pxT[lo:hi], "pyT": pyT[lo:hi]}
        else:
            m = {"fx": fx, "fy": fy, "pxT": pxT, "pyT": pyT}
        m.update(small)
        in_maps.append(m)
    return in_maps


def kernel(**inputs) -> np.ndarray:
    nc = get_program(SHARD)
    in_maps = make_in_maps(inputs, SHARD)
    res = run_bass_kernel_spmd(nc, in_maps, core_ids=list(range(N_CORES)))
    out = np.asarray(res.results[0]["out"], dtype=np.float32)
    return out[None]



# revision 5
# speedup vs baseline: 1.9719x; 1.9719x over previous
"""Trainium2 Bass kernel for nn_ExpandedResolventFMNet.

Mathematical reformulation (validated in fp64/fp16 against the jax reference):

The reference builds kron(A.T, My) [8192x4096], its Gram [4096^2], resolvent
kron masks, and solves a dense 4096x4096 system.  All of that collapses:

  first        = kron(A A^T, G),              G = My^T My
  second       = kron-sum of 64x64 factors; with X = Mx W the full system is
  M(W)         = S~ W G + LMBDA * sum_d Dd*( (Dd*W) G ) = R~    (* = Hadamard)
  S~           = Mx^T (A A^T) Mx
  R~           = G By A^T Mx,   By = Py fy
  Dd           = resolvent-mask difference matrices (64x64)
  output C     = (Mx W)^T

The device runs the transposed system Y = W^T:

  M'(Y) = G Y S~ + sum_d DdT * (G (DdT * Y)),   C = Y Mx^T

solved by PCG with the exact-kron preconditioner P^-1 x = Gi x Si, where
Gi, Si come from on-device Newton-Schulz iteration.  G's symmetry makes
every matmul transpose-free.

This version is fully unsharded: every core redundantly computes the whole
answer, so there are no collectives (the SPMD launch skew made the barrier +
two AllReduce cost ~64us on the measured core).  All matmuls run in fp16
(single-pass, 1 cycle/row vs fp32's split LOW/HIGH 2-pass) with fp32 PSUM
accumulation; CG state vectors and dots stay fp32.  End-to-end rel-err vs
the fp32 reference is ~2e-3 (gate 2e-2).  Inputs are cast to fp16 on the
host, which also halves HBM traffic; the DMA layout keeps each partition's
data contiguous in DRAM (125 descriptors/tensor instead of 5000).  The
V-contraction projections are emitted into the PE-queue gaps of the serial
Newton-Schulz chains so the tensor engine never idles on them.
"""

import numpy as np

import concourse.bacc as bacc
import concourse.mybir as mybir
from concourse.bass_isa import ReduceOp
from concourse.bass_utils import run_bass_kernel_spmd
from concourse.masks import make_identity
from concourse.tile import TileContext

F32 = mybir.dt.float32
F16 = mybir.dt.float16
K = 64          # spectral basis size
C = 128         # feature channels
V = 5000        # vertices
P = 125         # DMA partition rows (V = P * NB)
NB = 40         # contraction chunks
N_CORES = 8
N_ITERS = 7
NEWTON_STEPS_S = 6
NEWTON_STEPS_G = 4
SQRT_LMBDA = 10.0

SHARD = False   # kept for test.py compat; only the unsharded path exists

_PROGRAM_CACHE = {}


def build_program(shard: bool):
    nc = bacc.Bacc("TRN2", num_devices=N_CORES)

    fx_d = nc.dram_tensor("fx", [V, C], F16, kind="ExternalInput")
    fy_d = nc.dram_tensor("fy", [V, C], F16, kind="ExternalInput")
    pxT_d = nc.dram_tensor("pxT", [V, K], F16, kind="ExternalInput")
    pyT_d = nc.dram_tensor("pyT", [V, K], F16, kind="ExternalInput")
    # mx|my|mxT [64, 3*64] fp32 (host-concatenated)
    small_d = nc.dram_tensor("small", [K, 3 * K], F32, kind="ExternalInput")
    ev_d = nc.dram_tensor("ev", [1, 2 * K], F32, kind="ExternalInput")
    out_d = nc.dram_tensor("out", [K, K], F32, kind="ExternalOutput")

    fx_ap = fx_d.rearrange("(p n) c -> p n c", p=P)
    fy_ap = fy_d.rearrange("(p n) c -> p n c", p=P)
    pxT_ap = pxT_d.rearrange("(p n) c -> p n c", p=P)
    pyT_ap = pyT_d.rearrange("(p n) c -> p n c", p=P)

    with TileContext(nc) as tc:
        with (
            tc.tile_pool(name="big", bufs=1) as bp,
            tc.tile_pool(name="persist", bufs=1) as sp,
            tc.tile_pool(name="work", bufs=2) as wp,
            tc.tile_pool(name="psum", bufs=2, space="PSUM") as pp,
        ):
            _ps_state = {"i": 0}

            def ps_tile(shape):
                i = _ps_state["i"]
                _ps_state["i"] += 1
                return pp.tile(shape, F32, tag=f"ps{i % 3}", name=f"pst{i}")

            # ---------------- input DMA ------------------------------------
            # x-side + smalls on the SP queue (critical path), y-side issued
            # from the scalar engine so both sides stream concurrently.
            small_t = sp.tile([K, 3 * K], F32)
            ev_t = sp.tile([1, 2 * K], F32)
            fxh = bp.tile([P, NB, C], F16)
            fyh = bp.tile([P, NB, C], F16)
            pxh = bp.tile([P, NB, K], F16)
            pyh = bp.tile([P, NB, K], F16)
            H = NB // 2
            nc.sync.dma_start(small_t, small_d[:, :])
            nc.sync.dma_start(pxh[:, 0:H, :], pxT_ap[:, 0:H, :])
            nc.sync.dma_start(fxh[:, 0:H, :], fx_ap[:, 0:H, :])
            nc.sync.dma_start(ev_t, ev_d[:, :])
            nc.sync.dma_start(pxh[:, H:NB, :], pxT_ap[:, H:NB, :])
            nc.sync.dma_start(fxh[:, H:NB, :], fx_ap[:, H:NB, :])

            mx_s = small_t[:, 0:K]
            my_s = small_t[:, K:2 * K]
            mxT_s = small_t[:, 2 * K:3 * K]

            # fp16 copies of the small matrices (scalar engine), then the
            # y-side DMA issues (y data is needed ~8us later than x)
            m16 = sp.tile([K, 3 * K], F16)
            nc.scalar.copy(m16[:, 0:K], mx_s)
            nc.scalar.copy(m16[:, K:2 * K], my_s)
            nc.scalar.copy(m16[:, 2 * K:3 * K], mxT_s)
            mx16 = m16[:, 0:K]
            my16 = m16[:, K:2 * K]
            mxT16 = m16[:, 2 * K:3 * K]
            nc.scalar.dma_start(pyh[:, 0:H, :], pyT_ap[:, 0:H, :])
            nc.scalar.dma_start(fyh[:, 0:H, :], fy_ap[:, 0:H, :])
            nc.scalar.dma_start(pyh[:, H:NB, :], pyT_ap[:, H:NB, :])
            nc.scalar.dma_start(fyh[:, H:NB, :], fy_ap[:, H:NB, :])

            ident = sp.tile([C, C], F32)
            make_identity(nc, ident)
            id64 = ident[0:K, 0:K]
            ones_row = sp.tile([1, K], F32)
            nc.vector.memset(ones_row, 1.0)

            # ---------------- G = My^T My (early; data lands first) --------
            g_p = ps_tile([K, K])
            nc.tensor.matmul(g_p, my16, my16)
            g16 = sp.tile([K, K], F16, tag="g16", name="g16")
            nc.vector.tensor_copy(g16, g_p)

            # resolvent scalars: ev = [ex | ey]; t = ev/max(ev); im = 1/(1+t)
            # re = sqrt(t)*im; both scaled by sqrt(LMBDA).  (cheap DVE/ACT
            # work; the mask matrices themselves are built later so their
            # matmuls don't block the in-order PE queue.)
            evmax = sp.tile([1, 1], F32)
            nc.vector.tensor_reduce(evmax, ev_t, mybir.AxisListType.X,
                                    mybir.AluOpType.max)
            evrec = sp.tile([1, 1], F32)
            nc.vector.reciprocal(evrec, evmax)
            t_t = sp.tile([1, 2 * K], F32)
            nc.vector.tensor_scalar_mul(t_t, ev_t, evrec)
            tp1 = sp.tile([1, 2 * K], F32)
            nc.vector.tensor_scalar_add(tp1, t_t, 1.0)
            im_t = sp.tile([1, 2 * K], F32)
            nc.vector.reciprocal(im_t, tp1)
            sq_t = sp.tile([1, 2 * K], F32)
            nc.scalar.sqrt(sq_t, t_t)
            re_t = sp.tile([1, 2 * K], F32)
            nc.vector.tensor_mul(re_t, sq_t, im_t)
            nc.vector.tensor_scalar_mul(re_t, re_t, SQRT_LMBDA)
            nc.vector.tensor_scalar_mul(im_t, im_t, SQRT_LMBDA)

            # Newton-Schulz inverse (S symmetric PD): X' = 2X - X S X.
            # mat_p is the PSUM tile holding S; s16 its fp16 SBUF copy.
            # interleave(j) emits projection matmuls into the PE-queue gaps
            # left by the serial mm -> copy -> mm -> sub dependency chain.
            def newton_inverse(mat_p, s16, tag, steps, interleave=None):
                rs = sp.tile([K, 1], F32, tag=f"{tag}_rs", name=f"{tag}_rs")
                nc.vector.tensor_reduce(rs, mat_p, mybir.AxisListType.X,
                                        mybir.AluOpType.add,
                                        apply_absolute_value=True)
                nc.gpsimd.partition_all_reduce(rs, rs, K, ReduceOp.max)
                al = sp.tile([K, 1], F32, tag=f"{tag}_al", name=f"{tag}_al")
                nc.vector.reciprocal(al, rs)
                x_s = sp.tile([K, K], F16, tag=f"{tag}_x0", name=f"{tag}_x0")
                nc.vector.tensor_scalar_mul(x_s, id64, al)
                it_i = 0
                for it in range(steps):
                    t1 = ps_tile([K, K])
                    nc.tensor.matmul(t1, s16, x_s)            # S X (S sym)
                    if interleave is not None:
                        interleave(it_i); it_i += 1
                    t1s = wp.tile([K, K], F16, tag=f"{tag}_t1s",
                                  name=f"{tag}_t1s")
                    nc.vector.tensor_copy(t1s, t1)
                    t2 = ps_tile([K, K])
                    nc.tensor.matmul(t2, x_s, t1s)            # X (S X) (X sym)
                    if interleave is not None:
                        interleave(it_i); it_i += 1
                    xn = sp.tile([K, K], F16, tag=f"{tag}_x{it + 1}",
                                 name=f"{tag}_x{it + 1}")
                    nc.vector.scalar_tensor_tensor(
                        xn, x_s, 2.0, t2,
                        op0=mybir.AluOpType.mult,
                        op1=mybir.AluOpType.subtract)
                    x_s = xn
                if interleave is not None:
                    interleave(1000)   # flush any remainder
                return x_s

            # ---- x projections interleaved into Newton-G's PE gaps --------
            with tc.tile_pool(name="pacc", bufs=1, space="PSUM") as pacc:
                at_p = pacc.tile([C, K], F32)    # A^T  [C,K]
                byt_p = pacc.tile([C, K], F32)   # By^T [C,K]

                xprog = {"n": 0}

                def emit_xproj(upto):
                    while xprog["n"] < min(upto, NB):
                        n = xprog["n"]
                        nc.tensor.matmul(at_p, fxh[:, n, :], pxh[:, n, :],
                                         start=(n == 0), stop=(n == NB - 1))
                        xprog["n"] += 1

                gi16 = newton_inverse(
                    g_p, g16, "gi", NEWTON_STEPS_G,
                    interleave=lambda j: emit_xproj(j * 6))
                emit_xproj(NB)

                at16 = sp.tile([C, K], F16, tag="at16", name="at16")
                nc.vector.tensor_copy(at16, at_p)

                # S~ = Mx^T (A A^T) Mx
                sa_p = ps_tile([K, K])
                nc.tensor.matmul(sa_p, at16, at16)          # A A^T
                sa16 = wp.tile([K, K], F16, tag="sa16", name="sa16")
                nc.scalar.copy(sa16, sa_p)
                h1_p = ps_tile([K, K])
                nc.tensor.matmul(h1_p, sa16, mx16)          # S_A Mx (sym)
                h16 = wp.tile([K, K], F16, tag="h16", name="h16")
                nc.scalar.copy(h16, h1_p)
                st_p = ps_tile([K, K])
                nc.tensor.matmul(st_p, mx16, h16)           # Mx^T S_A Mx
                st16 = sp.tile([K, K], F16, tag="st16", name="st16")
                nc.scalar.copy(st16, st_p)

                # ---- y projections interleaved into Newton-S's gaps -------
                yprog = {"n": 0}

                def emit_yproj(upto):
                    while yprog["n"] < min(upto, NB):
                        n = yprog["n"]
                        nc.tensor.matmul(byt_p, fyh[:, n, :], pyh[:, n, :],
                                         start=(n == 0), stop=(n == NB - 1))
                        yprog["n"] += 1

                si16 = newton_inverse(
                    st_p, st16, "si", NEWTON_STEPS_S,
                    interleave=lambda j: emit_yproj((j + 1) * 4))
                emit_yproj(NB)

                byt16 = sp.tile([C, K], F16, tag="byt16", name="byt16")
                nc.vector.tensor_copy(byt16, byt_p)

            # D1T[a,i] = re2[a] - re1[i]; D2T likewise from im (emitted late
            # so the tiny mask matmuls never stall the PE queue; needed only
            # by the first CG matvec)
            d12t_s = sp.tile([K, 2 * K], F32)
            for idx, src in enumerate((re_t, im_t)):
                pa = ps_tile([K, K])
                nc.tensor.matmul(pa, src[0:1, K:2 * K], ones_row)  # v2[p]
                pb = ps_tile([K, K])
                nc.tensor.matmul(pb, ones_row, src[0:1, 0:K])      # v1[f]
                ta = wp.tile([K, K], F32, tag=f"dta{idx}", name=f"dta{idx}")
                nc.vector.tensor_copy(ta, pa)
                nc.vector.tensor_sub(d12t_s[:, idx * K:(idx + 1) * K], ta, pb)
            d1t_s = d12t_s[:, 0:K]
            d2t_s = d12t_s[:, K:2 * K]

            # ---- RHS' = G By A^T Mx (3 matmuls, G-symmetry trick) ---------
            byat_p = ps_tile([K, K])
            nc.tensor.matmul(byat_p, byt16, at16)       # By A^T
            byat16 = wp.tile([K, K], F16, tag="byat16", name="byat16")
            nc.scalar.copy(byat16, byat_p)
            s2_p = ps_tile([K, K])
            nc.tensor.matmul(s2_p, byat16, g16)         # (G ByA^T)^T (G sym)
            s2c = wp.tile([K, K], F16, tag="s2c", name="s2c")
            nc.scalar.copy(s2c, s2_p)
            rp_p = ps_tile([K, K])
            nc.tensor.matmul(rp_p, s2c, mx16)           # G ByA^T Mx
            r_s = sp.tile([K, K], F32)                  # CG residual (fp32)
            nc.vector.tensor_copy(r_s, rp_p)
            r16 = wp.tile([K, K], F16, tag="x16", name="r16i")
            nc.scalar.copy(r16, rp_p)

            # ------- PCG: pipelined (vector recurrences, exact dots) -------
            y_s = sp.tile([K, K], F32)
            nc.vector.memset(y_s, 0.0)
            p_s = sp.tile([K, K], F32)
            q_s = sp.tile([K, K], F32)
            s_s = sp.tile([K, K], F32)
            z_s = sp.tile([K, K], F32)
            u16 = sp.tile([K, 2 * K], F16)   # stacked [D1T*z | D2T*z] fp16

            def precond_psum(x16, tag):
                """P^-1 x in PSUM via (Gi x)^T = mm(lhsT=x16, rhs=Gi)."""
                ut_p = ps_tile([K, K])
                nc.tensor.matmul(ut_p, x16, gi16)
                ut16 = wp.tile([K, K], F16, tag=f"{tag}_ut", name=f"{tag}_ut")
                nc.scalar.copy(ut16, ut_p)
                v_p = ps_tile([K, K])
                nc.tensor.matmul(v_p, ut16, si16)
                return v_p

            def matvec_z(z16, tag):
                """w = M z into fp32 SBUF (also reads z_s for the masks)."""
                nc.vector.tensor_mul(u16[:, 0:K], d1t_s, z_s)
                nc.vector.tensor_mul(u16[:, K:2 * K], d2t_s, z_s)
                gzt_p = ps_tile([K, K])
                nc.tensor.matmul(gzt_p, z16, g16)         # (G z)^T
                gzt16 = wp.tile([K, K], F16, tag="mv_gzt", name="mv_gzt")
                nc.scalar.copy(gzt16, gzt_p)
                gu_p = ps_tile([K, 3 * K])
                nc.tensor.matmul(gu_p[:, 0:2 * K], g16, u16)   # G u (both)
                nc.tensor.matmul(gu_p[:, 2 * K:3 * K], gzt16, st16)  # (Gz)S~
                mm_s = wp.tile([K, 2 * K], F32, tag="mv_mm", name="mv_mm")
                nc.vector.tensor_mul(mm_s, d12t_s, gu_p[:, 0:2 * K])  # mask
                a1_s = wp.tile([K, K], F32, tag="mv_a1", name="mv_a1")
                nc.vector.tensor_add(a1_s, mm_s[:, 0:K], mm_s[:, K:2 * K])
                w_s = wp.tile([K, K], F32, tag="mv_w", name="mv_w")
                nc.vector.tensor_add(w_s, a1_s, gu_p[:, 2 * K:3 * K])
                w16 = wp.tile([K, K], F16, tag="x16", name=f"{tag}_w16")
                nc.scalar.copy(w16, w_s)
                return w_s, w16

            def dot_b(a_ap, b_ap, tag):
                """<a,b> broadcast to all partitions as [K,1] SBUF."""
                prod = wp.tile([K, K], F32, tag="dot_dm", name="dot_dm")
                acc = wp.tile([K, 1], F32, tag=f"{tag}_acc", name=f"{tag}_acc")
                nc.vector.scalar_tensor_tensor(
                    prod, a_ap, 1.0, b_ap,
                    op0=mybir.AluOpType.bypass, op1=mybir.AluOpType.mult,
                    accum_out=acc)
                nc.gpsimd.partition_all_reduce(acc, acc, K, ReduceOp.add)
                return acc

            # init: z = P^-1 r; w = Mz; v = P^-1 w; p=z, q=w, s=v
            z0_p = precond_psum(r16, "pcz")
            nc.vector.tensor_copy(z_s, z0_p)
            nc.vector.tensor_copy(p_s, z0_p)
            z16 = wp.tile([K, K], F16, tag="z16", name="z16i")
            nc.scalar.copy(z16, z0_p)
            rz0 = dot_b(r_s, z_s, "rz")
            rzrec = wp.tile([K, 1], F32, tag="rzrec", name="rzrec")
            nc.vector.reciprocal(rzrec, rz0)
            rzneg = wp.tile([K, 1], F32, tag="rzneg", name="rzneg")
            nc.vector.tensor_scalar_mul(rzneg, rz0, -1.0)
            w_s, w16 = matvec_z(z16, "init")
            nc.vector.tensor_copy(q_s, w_s)
            v_p = precond_psum(w16, "pcv")
            nc.vector.tensor_copy(s_s, v_p)

            for it in range(N_ITERS):
                # ---- alpha = rz/<p,q>; r,z,y updates ----
                pq = dot_b(p_s, q_s, "pq")
                pqr = wp.tile([K, 1], F32, tag="pqr", name="pqr")
                nc.vector.reciprocal(pqr, pq)
                if it < N_ITERS - 1:
                    an = wp.tile([K, 1], F32, tag="an", name="an")
                    nc.scalar.mul(an, rzneg, pqr)
                    nc.vector.scalar_tensor_tensor(
                        r_s, q_s, an, r_s,
                        op0=mybir.AluOpType.mult, op1=mybir.AluOpType.add)
                    nc.vector.scalar_tensor_tensor(
                        z_s, s_s, an, z_s,
                        op0=mybir.AluOpType.mult, op1=mybir.AluOpType.add)
                    z16 = wp.tile([K, K], F16, tag="z16", name=f"z16_{it}")
                    nc.scalar.copy(z16, z_s)
                al = wp.tile([K, 1], F32, tag="al", name="al")
                nc.scalar.mul(al, rz0, pqr)
                nc.vector.scalar_tensor_tensor(
                    y_s, p_s, al, y_s,
                    op0=mybir.AluOpType.mult, op1=mybir.AluOpType.add)

                if it == N_ITERS - 1:
                    break

                # ---- rz_new, beta; w/v for the NEXT q,s updates ----
                rz_new = dot_b(r_s, z_s, "rz")
                w_s, w16 = matvec_z(z16, f"i{it}")
                if it < N_ITERS - 2:
                    v_p = precond_psum(w16, "pcv")
                bt = wp.tile([K, 1], F32, tag="bt", name="bt")
                nc.vector.tensor_mul(bt, rz_new, rzrec)
                nc.vector.scalar_tensor_tensor(
                    p_s, p_s, bt, z_s,
                    op0=mybir.AluOpType.mult, op1=mybir.AluOpType.add)
                nc.vector.scalar_tensor_tensor(
                    q_s, q_s, bt, w_s,
                    op0=mybir.AluOpType.mult, op1=mybir.AluOpType.add)
                if it < N_ITERS - 2:
                    nc.vector.scalar_tensor_tensor(
                        s_s, s_s, bt, v_p,
                        op0=mybir.AluOpType.mult, op1=mybir.AluOpType.add)
                rz0 = rz_new
                rzrec = wp.tile([K, 1], F32, tag="rzrec", name="rzrec")
                nc.vector.reciprocal(rzrec, rz0)
                rzneg = wp.tile([K, 1], F32, tag="rzneg", name="rzneg")
                nc.vector.tensor_scalar_mul(rzneg, rz0, -1.0)

            # ---------------- output: C = Y Mx^T ---------------------------
            yt_p = ps_tile([K, K])
            nc.tensor.transpose(yt_p, y_s, id64)
            yt16 = wp.tile([K, K], F16, tag="yt16", name="yt16")
            nc.scalar.copy(yt16, yt_p)
            c_p = ps_tile([K, K])
            nc.tensor.matmul(c_p, yt16, mxT16)          # Y Mx^T
            c_s = wp.tile([K, K], F32, tag="c_s", name="c_s")
            nc.vector.tensor_copy(c_s, c_p)
            nc.sync.dma_start(out_d[:, :], c_s)

    nc.finalize()
    return nc


def get_program(shard: bool = False):
    if shard not in _PROGRAM_CACHE:
        _PROGRAM_CACHE[shard] = build_program(shard)
    return _PROGRAM_CACHE[shard]


def make_in_maps(inputs, shard: bool = False):
    fx = np.asarray(inputs["feat_x"], np.float32)[0].astype(np.float16)
    fy = np.asarray(inputs["feat_y"], np.float32)[0].astype(np.float16)
    pxT = np.ascontiguousarray(
        np.asarray(inputs["evecs_trans_x"], np.float32)[0].T).astype(np.float16)
    pyT = np.ascontiguousarray(
        np.asarray(inputs["evecs_trans_y"], np.float32)[0].T).astype(np.float16)
    mx = np.asarray(inputs["sqrtMk_x"], np.float32)[0]
    my = np.asarray(inputs["sqrtMk_y"], np.float32)[0]
    small = np.ascontiguousarray(np.concatenate([mx, my, mx.T], axis=1))
    ev = np.ascontiguousarray(np.concatenate([
        np.asarray(inputs["evals_x"], np.float32)[0],
        np.asarray(inputs["evals_y"], np.float32)[0],
    ])[None, :])
    m = {"fx": fx, "fy": fy, "pxT": pxT, "pyT": pyT,
         "small": small, "ev": ev}
    return [dict(m) for _ in range(N_CORES)]


def kernel(**inputs) -> np.ndarray:
    nc = get_program(SHARD)
    in_maps = make_in_maps(inputs, SHARD)
    res = run_bass_kernel_spmd(nc, in_maps, core_ids=list(range(N_CORES)))
    out = np.asarray(res.results[0]["out"], dtype=np.float32)
    return out[None]


# revision 7
# speedup vs baseline: 2.0927x; 1.0613x over previous
"""Trainium2 Bass kernel for nn_ExpandedResolventFMNet.

Mathematical reformulation (validated in fp64/fp16 against the jax reference):

The reference builds kron(A.T, My) [8192x4096], its Gram [4096^2], resolvent
kron masks, and solves a dense 4096x4096 system.  All of that collapses:

  first        = kron(A A^T, G),              G = My^T My
  second       = kron-sum of 64x64 factors; with X = Mx W the full system is
  M(W)         = S~ W G + LMBDA * sum_d Dd*( (Dd*W) G ) = R~    (* = Hadamard)
  S~           = Mx^T (A A^T) Mx
  R~           = G By A^T Mx,   By = Py fy
  Dd           = resolvent-mask difference matrices (64x64)
  output C     = (Mx W)^T

The device runs the transposed system Y = W^T:

  M'(Y) = G Y S~ + sum_d DdT * (G (DdT * Y)),   C = Y Mx^T

solved by PCG with the exact-kron preconditioner P^-1 x = Gi x Si, where
Gi, Si come from on-device Newton-Schulz iteration.  G's symmetry makes
every matmul transpose-free.

Fully unsharded: every core redundantly computes the whole answer, so there
are no collectives (SPMD launch skew made the barrier + two AllReduce cost
~64us on the measured core).  All matmuls run in fp16 (single-pass, 1
cycle/row vs fp32's split LOW_HIGH 2-pass) with fp32 PSUM accumulation; CG
state stays fp32 except the search direction p (fp16).  Inputs are cast to
fp16 on the host (halves HBM traffic); the DMA layout keeps each
partition's data contiguous in DRAM.  HWDGE queues stripe over a shared
5-SDMA-engine pool (~135 GB/s), so the critical x-side tensors are split
across both HWDGE queues while the y-side goes through gpsimd SWDGE.  CG
keeps the vector engine lean: r/z and q/s updates are fused into [64,128]
tiles, and y accumulates in PSUM via matmul with an alpha-scaled identity.
"""

import numpy as np

import concourse.bacc as bacc
import concourse.mybir as mybir
from concourse.bass_isa import ReduceOp
from concourse.bass_utils import run_bass_kernel_spmd
from concourse.masks import make_identity
from concourse.tile import TileContext

F32 = mybir.dt.float32
F16 = mybir.dt.float16
K = 64          # spectral basis size
C = 128         # feature channels
V = 5000        # vertices
P = 125         # DMA partition rows (V = P * NB)
NB = 40         # contraction chunks
N_CORES = 8
N_ITERS = 6
NEWTON_STEPS_S = 6
NEWTON_STEPS_G = 4
SQRT_LMBDA = 10.0

SHARD = False   # kept for test.py compat; only the unsharded path exists

_PROGRAM_CACHE = {}


def build_program(shard: bool):
    nc = bacc.Bacc("TRN2", num_devices=N_CORES)

    fx_d = nc.dram_tensor("fx", [V, C], F16, kind="ExternalInput")
    fy_d = nc.dram_tensor("fy", [V, C], F16, kind="ExternalInput")
    pxT_d = nc.dram_tensor("pxT", [V, K], F16, kind="ExternalInput")
    pyT_d = nc.dram_tensor("pyT", [V, K], F16, kind="ExternalInput")
    # mx|my|mxT [64, 3*64] fp32 (host-concatenated)
    small_d = nc.dram_tensor("small", [K, 3 * K], F32, kind="ExternalInput")
    ev_d = nc.dram_tensor("ev", [1, 2 * K], F32, kind="ExternalInput")
    out_d = nc.dram_tensor("out", [K, K], F32, kind="ExternalOutput")

    fx_ap = fx_d.rearrange("(p n) c -> p n c", p=P)
    fy_ap = fy_d.rearrange("(p n) c -> p n c", p=P)
    pxT_ap = pxT_d.rearrange("(p n) c -> p n c", p=P)
    pyT_ap = pyT_d.rearrange("(p n) c -> p n c", p=P)

    with TileContext(nc) as tc:
        with (
            tc.tile_pool(name="big", bufs=1) as bp,
            tc.tile_pool(name="persist", bufs=1) as sp,
            tc.tile_pool(name="work", bufs=2) as wp,
            tc.tile_pool(name="psum", bufs=2, space="PSUM") as pp,
            tc.tile_pool(name="yacc", bufs=1, space="PSUM") as yp,
        ):
            _ps_state = {"i": 0}

            def ps_tile(shape):
                i = _ps_state["i"]
                _ps_state["i"] += 1
                return pp.tile(shape, F32, tag=f"ps{i % 2}", name=f"pst{i}")

            # ---------------- input DMA ------------------------------------
            # x-side is the critical path: split it over both HWDGE queues
            # (sync carries fx, scalar carries pxT).  y-side goes through
            # gpsimd SWDGE so it streams concurrently without stealing the
            # HWDGE ring.  smalls first (they gate G and the fp16 copies).
            small_t = sp.tile([K, 3 * K], F32)
            ev_t = sp.tile([1, 2 * K], F32)
            fxh = bp.tile([P, NB, C], F16)
            fyh = bp.tile([P, NB, C], F16)
            pxh = bp.tile([P, NB, K], F16)
            pyh = bp.tile([P, NB, K], F16)
            H = NB // 2
            nc.sync.dma_start(small_t, small_d[:, :])
            nc.sync.dma_start(fxh[:, 0:H, :], fx_ap[:, 0:H, :])
            nc.sync.dma_start(ev_t, ev_d[:, :])
            nc.sync.dma_start(fxh[:, H:NB, :], fx_ap[:, H:NB, :])
            nc.scalar.dma_start(pxh[:, 0:H, :], pxT_ap[:, 0:H, :])
            nc.scalar.dma_start(pxh[:, H:NB, :], pxT_ap[:, H:NB, :])
            nc.gpsimd.dma_start(pyh[:, 0:H, :], pyT_ap[:, 0:H, :])
            nc.gpsimd.dma_start(fyh[:, 0:H, :], fy_ap[:, 0:H, :])
            nc.gpsimd.dma_start(pyh[:, H:NB, :], pyT_ap[:, H:NB, :])
            nc.gpsimd.dma_start(fyh[:, H:NB, :], fy_ap[:, H:NB, :])

            mx_s = small_t[:, 0:K]
            my_s = small_t[:, K:2 * K]
            mxT_s = small_t[:, 2 * K:3 * K]

            # fp16 copies of the small matrices (scalar engine)
            m16 = sp.tile([K, 3 * K], F16)
            nc.scalar.copy(m16[:, 0:K], mx_s)
            nc.scalar.copy(m16[:, K:2 * K], my_s)
            nc.scalar.copy(m16[:, 2 * K:3 * K], mxT_s)
            mx16 = m16[:, 0:K]
            my16 = m16[:, K:2 * K]
            mxT16 = m16[:, 2 * K:3 * K]

            ident = sp.tile([C, C], F32)
            make_identity(nc, ident)
            id64 = ident[0:K, 0:K]
            id16 = sp.tile([K, K], F16)
            nc.scalar.copy(id16, id64)
            ones_row = sp.tile([1, K], F32)
            nc.vector.memset(ones_row, 1.0)

            # ---------------- G = My^T My (early; data lands first) --------
            g_p = ps_tile([K, K])
            nc.tensor.matmul(g_p, my16, my16)
            g16 = sp.tile([K, K], F16, tag="g16", name="g16")
            nc.vector.tensor_copy(g16, g_p)

            # resolvent scalars: ev = [ex | ey]; t = ev/max(ev); im = 1/(1+t)
            # re = sqrt(t)*im; both scaled by sqrt(LMBDA)
            evmax = sp.tile([1, 1], F32)
            nc.vector.tensor_reduce(evmax, ev_t, mybir.AxisListType.X,
                                    mybir.AluOpType.max)
            evrec = sp.tile([1, 1], F32)
            nc.vector.reciprocal(evrec, evmax)
            t_t = sp.tile([1, 2 * K], F32)
            nc.vector.tensor_scalar_mul(t_t, ev_t, evrec)
            tp1 = sp.tile([1, 2 * K], F32)
            nc.vector.tensor_scalar_add(tp1, t_t, 1.0)
            im_t = sp.tile([1, 2 * K], F32)
            nc.vector.reciprocal(im_t, tp1)
            sq_t = sp.tile([1, 2 * K], F32)
            nc.scalar.sqrt(sq_t, t_t)
            re_t = sp.tile([1, 2 * K], F32)
            nc.vector.tensor_mul(re_t, sq_t, im_t)
            nc.vector.tensor_scalar_mul(re_t, re_t, SQRT_LMBDA)
            nc.vector.tensor_scalar_mul(im_t, im_t, SQRT_LMBDA)

            # Newton-Schulz inverse (S symmetric PD): X' = 2X - X S X.
            # interleave(j) emits projection matmuls into the PE-queue gaps.
            def newton_inverse(mat_p, s16, tag, steps, interleave=None):
                rs = sp.tile([K, 1], F32, tag=f"{tag}_rs", name=f"{tag}_rs")
                nc.vector.tensor_reduce(rs, mat_p, mybir.AxisListType.X,
                                        mybir.AluOpType.add,
                                        apply_absolute_value=True)
                nc.gpsimd.partition_all_reduce(rs, rs, K, ReduceOp.max)
                al = sp.tile([K, 1], F32, tag=f"{tag}_al", name=f"{tag}_al")
                nc.vector.reciprocal(al, rs)
                x_s = sp.tile([K, K], F16, tag=f"{tag}_x0", name=f"{tag}_x0")
                nc.vector.tensor_scalar_mul(x_s, id64, al)
                it_i = 0
                for it in range(steps):
                    t1 = ps_tile([K, K])
                    nc.tensor.matmul(t1, s16, x_s)            # S X (S sym)
                    if interleave is not None:
                        interleave(it_i); it_i += 1
                    t1s = wp.tile([K, K], F16, tag=f"{tag}_t1s",
                                  name=f"{tag}_t1s")
                    nc.vector.tensor_copy(t1s, t1)
                    t2 = ps_tile([K, K])
                    nc.tensor.matmul(t2, x_s, t1s)            # X (S X) (X sym)
                    if interleave is not None:
                        interleave(it_i); it_i += 1
                    xn = sp.tile([K, K], F16, tag=f"{tag}_x{it + 1}",
                                 name=f"{tag}_x{it + 1}")
                    nc.vector.scalar_tensor_tensor(
                        xn, x_s, 2.0, t2,
                        op0=mybir.AluOpType.mult,
                        op1=mybir.AluOpType.subtract)
                    x_s = xn
                if interleave is not None:
                    interleave(1000)   # flush any remainder
                return x_s

            # ---- x projections interleaved into Newton-G's PE gaps --------
            with tc.tile_pool(name="pacc", bufs=1, space="PSUM") as pacc:
                at_p = pacc.tile([C, K], F32)    # A^T  [C,K]
                byt_p = pacc.tile([C, K], F32)   # By^T [C,K]

                xprog = {"n": 0}

                def emit_xproj(upto):
                    while xprog["n"] < min(upto, NB):
                        n = xprog["n"]
                        nc.tensor.matmul(at_p, fxh[:, n, :], pxh[:, n, :],
                                         start=(n == 0), stop=(n == NB - 1))
                        xprog["n"] += 1

                gi16 = newton_inverse(
                    g_p, g16, "gi", NEWTON_STEPS_G,
                    interleave=lambda j: emit_xproj(j * 6))
                emit_xproj(NB)

                at16 = sp.tile([C, K], F16, tag="at16", name="at16")
                nc.vector.tensor_copy(at16, at_p)

                # S~ = Mx^T (A A^T) Mx
                sa_p = ps_tile([K, K])
                nc.tensor.matmul(sa_p, at16, at16)          # A A^T
                sa16 = wp.tile([K, K], F16, tag="sa16", name="sa16")
                nc.vector.tensor_copy(sa16, sa_p)
                h1_p = ps_tile([K, K])
                nc.tensor.matmul(h1_p, sa16, mx16)          # S_A Mx (sym)
                h16 = wp.tile([K, K], F16, tag="h16", name="h16")
                nc.vector.tensor_copy(h16, h1_p)
                st_p = ps_tile([K, K])
                nc.tensor.matmul(st_p, mx16, h16)           # Mx^T S_A Mx
                st16 = sp.tile([K, K], F16, tag="st16", name="st16")
                nc.scalar.copy(st16, st_p)

                # ---- y projections interleaved into Newton-S's gaps -------
                yprog = {"n": 0}

                def emit_yproj(upto):
                    while yprog["n"] < min(upto, NB):
                        n = yprog["n"]
                        nc.tensor.matmul(byt_p, fyh[:, n, :], pyh[:, n, :],
                                         start=(n == 0), stop=(n == NB - 1))
                        yprog["n"] += 1

                si16 = newton_inverse(
                    st_p, st16, "si", NEWTON_STEPS_S,
                    interleave=lambda j: emit_yproj((j + 1) * 4))
                emit_yproj(NB)

                byt16 = sp.tile([C, K], F16, tag="byt16", name="byt16")
                nc.vector.tensor_copy(byt16, byt_p)

            # D1T[a,i] = re2[a] - re1[i]; D2T likewise from im (emitted late
            # so the tiny mask matmuls never stall the PE queue)
            d12t_s = sp.tile([K, 2 * K], F32)
            for idx, src in enumerate((re_t, im_t)):
                pa = ps_tile([K, K])
                nc.tensor.matmul(pa, src[0:1, K:2 * K], ones_row)  # v2[p]
                pb = ps_tile([K, K])
                nc.tensor.matmul(pb, ones_row, src[0:1, 0:K])      # v1[f]
                ta = wp.tile([K, K], F32, tag=f"dta{idx}", name=f"dta{idx}")
                nc.vector.tensor_copy(ta, pa)
                nc.vector.tensor_sub(d12t_s[:, idx * K:(idx + 1) * K], ta, pb)
            d12v = d12t_s[:, :].rearrange("p (a b) -> p a b", a=2)

            # ---- RHS' = G By A^T Mx (3 matmuls, G-symmetry trick) ---------
            byat_p = ps_tile([K, K])
            nc.tensor.matmul(byat_p, byt16, at16)       # By A^T
            byat16 = wp.tile([K, K], F16, tag="byat16", name="byat16")
            nc.scalar.copy(byat16, byat_p)
            s2_p = ps_tile([K, K])
            nc.tensor.matmul(s2_p, byat16, g16)         # (G ByA^T)^T (G sym)
            s2c = wp.tile([K, K], F16, tag="s2c", name="s2c")
            nc.scalar.copy(s2c, s2_p)
            rp_p = ps_tile([K, K])
            nc.tensor.matmul(rp_p, s2c, mx16)           # G ByA^T Mx

            # ------- PCG state: fused tiles --------------------------------
            # rz_s = [r | z] fp32;  qs_s = [q | s] fp32;  wv_s = [w | v] fp32
            # p16 fp16 search direction; y accumulates in PSUM via matmul.
            rz_s = sp.tile([K, 2 * K], F32)
            qs_s = sp.tile([K, 2 * K], F32)
            wv_s = sp.tile([K, 2 * K], F32)
            u16 = sp.tile([K, 2 * K], F16)
            p16 = sp.tile([K, K], F16)
            y_p = yp.tile([K, K], F32)
            r_sl = rz_s[:, 0:K]
            z_sl = rz_s[:, K:2 * K]
            u16v = u16[:, :].rearrange("p (a b) -> p a b", a=2)
            z_bc = z_sl.rearrange("p (o b) -> p o b", o=1).broadcast_to(
                [K, 2, K])

            nc.vector.tensor_copy(r_sl, rp_p)
            r16 = wp.tile([K, K], F16, tag="x16", name="r16i")
            nc.scalar.copy(r16, rp_p)

            def precond_psum(x16, tag):
                """P^-1 x in PSUM via (Gi x)^T = mm(lhsT=x16, rhs=Gi)."""
                ut_p = ps_tile([K, K])
                nc.tensor.matmul(ut_p, x16, gi16)
                ut16 = wp.tile([K, K], F16, tag=f"{tag}_ut", name=f"{tag}_ut")
                nc.scalar.copy(ut16, ut_p)
                v_p = ps_tile([K, K])
                nc.tensor.matmul(v_p, ut16, si16)
                return v_p

            def matvec_z(z16, tag):
                """w = M z -> wv_s[:, 0:K]; w16 returned for the precond."""
                nc.vector.tensor_mul(u16v, d12v, z_bc)   # [D1T*z | D2T*z]
                gzt_p = ps_tile([K, K])
                nc.tensor.matmul(gzt_p, z16, g16)         # (G z)^T
                gzt16 = wp.tile([K, K], F16, tag="mv_gzt", name="mv_gzt")
                nc.scalar.copy(gzt16, gzt_p)
                gu_p = ps_tile([K, 3 * K])
                nc.tensor.matmul(gu_p[:, 0:2 * K], g16, u16)   # G u (both)
                nc.tensor.matmul(gu_p[:, 2 * K:3 * K], gzt16, st16)  # (Gz)S~
                mm_s = wp.tile([K, 2 * K], F32, tag="mv_mm", name="mv_mm")
                nc.vector.tensor_mul(mm_s, d12t_s, gu_p[:, 0:2 * K])  # mask
                a1_s = wp.tile([K, K], F32, tag="mv_a1", name="mv_a1")
                nc.vector.tensor_add(a1_s, mm_s[:, 0:K], mm_s[:, K:2 * K])
                nc.vector.tensor_add(wv_s[:, 0:K], a1_s, gu_p[:, 2 * K:3 * K])
                w16 = wp.tile([K, K], F16, tag="x16", name=f"{tag}_w16")
                nc.scalar.copy(w16, wv_s[:, 0:K])
                return w16

            def dot_b(a_ap, b_ap, tag):
                """<a,b> broadcast to all partitions as [K,1] SBUF."""
                prod = wp.tile([K, K], F32, tag="dot_dm", name="dot_dm")
                acc = wp.tile([K, 1], F32, tag=f"{tag}_acc", name=f"{tag}_acc")
                nc.vector.scalar_tensor_tensor(
                    prod, a_ap, 1.0, b_ap,
                    op0=mybir.AluOpType.bypass, op1=mybir.AluOpType.mult,
                    accum_out=acc)
                nc.gpsimd.partition_all_reduce(acc, acc, K, ReduceOp.add)
                return acc

            # init: z = P^-1 r; p = z; w = Mz; v = P^-1 w; q = w, s = v
            z0_p = precond_psum(r16, "pcz")
            nc.vector.tensor_copy(z_sl, z0_p)
            nc.scalar.copy(p16, z0_p)
            z16 = wp.tile([K, K], F16, tag="z16", name="z16i")
            nc.scalar.copy(z16, z0_p)
            rz0 = dot_b(r_sl, z_sl, "rz")
            rzrec = wp.tile([K, 1], F32, tag="rzrec", name="rzrec")
            nc.vector.reciprocal(rzrec, rz0)
            rzneg = wp.tile([K, 1], F32, tag="rzneg", name="rzneg")
            nc.vector.tensor_scalar_mul(rzneg, rz0, -1.0)
            w16 = matvec_z(z16, "init")
            nc.vector.tensor_copy(qs_s[:, 0:K], wv_s[:, 0:K])
            v_p = precond_psum(w16, "pcv")
            nc.scalar.copy(qs_s[:, K:2 * K], v_p)

            for it in range(N_ITERS):
                # ---- alpha = rz/<p,q>; fused [r|z] update; y += alpha p ----
                pq = dot_b(p16, qs_s[:, 0:K], "pq")
                pqr = wp.tile([K, 1], F32, tag="pqr", name="pqr")
                nc.vector.reciprocal(pqr, pq)
                if it < N_ITERS - 1:
                    an = wp.tile([K, 1], F32, tag="an", name="an")
                    nc.scalar.mul(an, rzneg, pqr)
                    nc.vector.scalar_tensor_tensor(
                        rz_s, qs_s, an, rz_s,
                        op0=mybir.AluOpType.mult, op1=mybir.AluOpType.add)
                    z16 = wp.tile([K, K], F16, tag="z16", name=f"z16_{it}")
                    nc.scalar.copy(z16, z_sl)
                al = wp.tile([K, 1], F32, tag="al", name="al")
                nc.scalar.mul(al, rz0, pqr)
                ida = wp.tile([K, K], F16, tag="ida", name="ida")
                nc.scalar.mul(ida, id16, al)              # alpha * I (fp16)
                nc.tensor.matmul(y_p, ida, p16,
                                 start=(it == 0), stop=(it == N_ITERS - 1))

                if it == N_ITERS - 1:
                    break

                # ---- rz_new, beta; w/v for the NEXT q,s updates ----
                rz_new = dot_b(r_sl, z_sl, "rz")
                w16 = matvec_z(z16, f"i{it}")
                if it < N_ITERS - 2:
                    v_p = precond_psum(w16, "pcv")
                    nc.scalar.copy(wv_s[:, K:2 * K], v_p)
                bt = wp.tile([K, 1], F32, tag="bt", name="bt")
                nc.vector.tensor_mul(bt, rz_new, rzrec)
                nc.vector.scalar_tensor_tensor(
                    p16, p16, bt, z_sl,
                    op0=mybir.AluOpType.mult, op1=mybir.AluOpType.add)
                if it < N_ITERS - 2:
                    nc.vector.scalar_tensor_tensor(
                        qs_s, qs_s, bt, wv_s,
                        op0=mybir.AluOpType.mult, op1=mybir.AluOpType.add)
                else:
                    nc.vector.scalar_tensor_tensor(
                        qs_s[:, 0:K], qs_s[:, 0:K], bt, wv_s[:, 0:K],
                        op0=mybir.AluOpType.mult, op1=mybir.AluOpType.add)
                rz0 = rz_new
                rzrec = wp.tile([K, 1], F32, tag="rzrec", name="rzrec")
                nc.vector.reciprocal(rzrec, rz0)
                rzneg = wp.tile([K, 1], F32, tag="rzneg", name="rzneg")
                nc.vector.tensor_scalar_mul(rzneg, rz0, -1.0)

            # ---------------- output: C = Y Mx^T ---------------------------
            y_s = sp.tile([K, K], F32)
            nc.vector.tensor_copy(y_s, y_p)
            yt_p = ps_tile([K, K])
            nc.tensor.transpose(yt_p, y_s, id64)
            yt16 = wp.tile([K, K], F16, tag="yt16", name="yt16")
            nc.scalar.copy(yt16, yt_p)
            c_p = ps_tile([K, K])
            nc.tensor.matmul(c_p, yt16, mxT16)          # Y Mx^T
            c_s = wp.tile([K, K], F32, tag="c_s", name="c_s")
            nc.vector.tensor_copy(c_s, c_p)
            nc.sync.dma_start(out_d[:, :], c_s)

    nc.finalize()
    return nc


def get_program(shard: bool = False):
    if shard not in _PROGRAM_CACHE:
        _PROGRAM_CACHE[shard] = build_program(shard)
    return _PROGRAM_CACHE[shard]


def make_in_maps(inputs, shard: bool = False):
    fx = np.asarray(inputs["feat_x"], np.float32)[0].astype(np.float16)
    fy = np.asarray(inputs["feat_y"], np.float32)[0].astype(np.float16)
    pxT = np.ascontiguousarray(
        np.asarray(inputs["evecs_trans_x"], np.float32)[0].T).astype(np.float16)
    pyT = np.ascontiguousarray(
        np.asarray(inputs["evecs_trans_y"], np.float32)[0].T).astype(np.float16)
    mx = np.asarray(inputs["sqrtMk_x"], np.float32)[0]
    my = np.asarray(inputs["sqrtMk_y"], np.float32)[0]
    small = np.ascontiguousarray(np.concatenate([mx, my, mx.T], axis=1))
    ev = np.ascontiguousarray(np.concatenate([
        np.asarray(inputs["evals_x"], np.float32)[0],
        np.asarray(inputs["evals_y"], np.float32)[0],
    ])[None, :])
    m = {"fx": fx, "fy": fy, "pxT": pxT, "pyT": pyT,
         "small": small, "ev": ev}
    return [dict(m) for _ in range(N_CORES)]


def kernel(**inputs) -> np.ndarray:
    nc = get_program(SHARD)
    in_maps = make_in_maps(inputs, SHARD)
    res = run_bass_kernel_spmd(nc, in_maps, core_ids=list(range(N_CORES)))
    out = np.asarray(res.results[0]["out"], dtype=np.float32)
    return out[None]


# revision 12
# speedup vs baseline: 2.4703x; 1.1804x over previous
"""Trainium2 Bass kernel for nn_ExpandedResolventFMNet.

Mathematical reformulation (validated in fp64/fp16 against the jax reference):

The reference builds kron(A.T, My) [8192x4096], its Gram [4096^2], resolvent
kron masks, and solves a dense 4096x4096 system.  All of that collapses:

  first        = kron(A A^T, G),              G = My^T My
  second       = kron-sum of 64x64 factors; with X = Mx W the full system is
  M(W)         = S~ W G + LMBDA * sum_d Dd*( (Dd*W) G ) = R~    (* = Hadamard)
  S~           = Mx^T (A A^T) Mx
  R~           = G By A^T Mx,   By = Py fy
  Dd           = resolvent-mask difference matrices (64x64)
  output C     = (Mx W)^T

The device runs the transposed system Y = W^T:

  M'(Y) = G Y S~ + sum_d DdT * (G (DdT * Y)),   C = Y Mx^T

solved by PCG with the exact-kron preconditioner P^-1 x = Gi x Si, where
Gi, Si come from on-device Newton-Schulz iteration (two-hop steps:
X' = X (2I - S X)).  G's symmetry makes every matmul transpose-free, and
Y^T is accumulated in PSUM via matmul against an alpha-scaled identity so
the output needs no transpose.

Fully unsharded: every core redundantly computes the whole answer, so there
are no collectives (SPMD launch skew made the barrier + two AllReduce cost
~64us on the measured core).  All matmuls run in fp16 (single-pass, 1
cycle/row vs fp32's split LOW_HIGH 2-pass) with fp32 PSUM accumulation; CG
state stays fp32 except the search direction p (fp16).  Inputs are cast to
fp16 on the host (halves HBM traffic).  The four big tensors stream through
the gpsimd SWDGE queue, which stripes descriptors over all 16 SDMA engines
(the HWDGE rings only get 5); queue FIFO order gives the x-side strict
priority.  Each partition's data is contiguous in DRAM (125 descriptors
per tensor).
"""

import numpy as np

import concourse.bacc as bacc
import concourse.mybir as mybir
from concourse.bass_isa import ReduceOp
from concourse.bass_utils import run_bass_kernel_spmd
from concourse.masks import make_identity
from concourse.tile import TileContext

F32 = mybir.dt.float32
F16 = mybir.dt.float16
K = 64          # spectral basis size
C = 128         # feature channels
V = 5000        # vertices
P = 125         # DMA partition rows (V = P * NB)
NB = 40         # contraction chunks
N_CORES = 8
N_ITERS = 6
NEWTON_STEPS_S = 6
NEWTON_STEPS_G = 4
SQRT_LMBDA = 10.0

SHARD = False   # kept for test.py compat; only the unsharded path exists

_PROGRAM_CACHE = {}


def build_program(shard: bool):
    nc = bacc.Bacc("TRN2", num_devices=N_CORES)

    fx_d = nc.dram_tensor("fx", [V, C], F16, kind="ExternalInput")
    fy_d = nc.dram_tensor("fy", [V, C], F16, kind="ExternalInput")
    pxT_d = nc.dram_tensor("pxT", [V, K], F16, kind="ExternalInput")
    pyT_d = nc.dram_tensor("pyT", [V, K], F16, kind="ExternalInput")
    # mx|my|mxT [64, 3*64] fp32 (host-concatenated)
    small_d = nc.dram_tensor("small", [K, 3 * K], F32, kind="ExternalInput")
    ev_d = nc.dram_tensor("ev", [1, 2 * K], F32, kind="ExternalInput")
    out_d = nc.dram_tensor("out", [K, K], F32, kind="ExternalOutput")

    fx_ap = fx_d.rearrange("(p n) c -> p n c", p=P)
    fy_ap = fy_d.rearrange("(p n) c -> p n c", p=P)
    pxT_ap = pxT_d.rearrange("(p n) c -> p n c", p=P)
    pyT_ap = pyT_d.rearrange("(p n) c -> p n c", p=P)

    with TileContext(nc) as tc:
        with (
            tc.tile_pool(name="big", bufs=1) as bp,
            tc.tile_pool(name="persist", bufs=1) as sp,
            tc.tile_pool(name="work", bufs=2) as wp,
            tc.tile_pool(name="psum", bufs=2, space="PSUM") as pp,
            tc.tile_pool(name="yacc", bufs=1, space="PSUM") as yp,
        ):
            _ps_state = {"i": 0}

            def ps_tile(shape):
                i = _ps_state["i"]
                _ps_state["i"] += 1
                return pp.tile(shape, F32, tag=f"ps{i % 2}", name=f"pst{i}")

            # ---------------- input DMA ------------------------------------
            # smalls ride the (otherwise idle) HWDGE queues; the four big
            # tensors stream through gpsimd SWDGE in x-first FIFO order.
            small_t = sp.tile([K, 3 * K], F32)
            ev_t = sp.tile([1, 2 * K], F32)
            fxh = bp.tile([P, NB, C], F16)
            fyh = bp.tile([P, NB, C], F16)
            pxh = bp.tile([P, NB, K], F16)
            pyh = bp.tile([P, NB, K], F16)
            nc.sync.dma_start(small_t, small_d[:, :])
            nc.sync.dma_start(ev_t, ev_d[:, :])
            nc.gpsimd.dma_start(pxh[:, :, :], pxT_ap[:, :, :])
            nc.gpsimd.dma_start(fxh[:, :, :], fx_ap[:, :, :])
            nc.gpsimd.dma_start(pyh[:, :, :], pyT_ap[:, :, :])
            nc.gpsimd.dma_start(fyh[:, :, :], fy_ap[:, :, :])

            mx_s = small_t[:, 0:K]
            my_s = small_t[:, K:2 * K]
            mxT_s = small_t[:, 2 * K:3 * K]

            # fp16 copies of the small matrices (scalar engine)
            m16 = sp.tile([K, 3 * K], F16)
            nc.scalar.copy(m16[:, 0:K], mx_s)
            nc.scalar.copy(m16[:, K:2 * K], my_s)
            nc.scalar.copy(m16[:, 2 * K:3 * K], mxT_s)
            mx16 = m16[:, 0:K]
            my16 = m16[:, K:2 * K]
            mxT16 = m16[:, 2 * K:3 * K]

            ident = sp.tile([C, C], F32)
            make_identity(nc, ident)
            id64 = ident[0:K, 0:K]
            id16 = sp.tile([K, K], F16)
            nc.scalar.copy(id16, id64)
            ones_row = sp.tile([1, K], F32)
            nc.vector.memset(ones_row, 1.0)

            # ---------------- G = My^T My (early; data lands first) --------
            g_p = ps_tile([K, K])
            nc.tensor.matmul(g_p, my16, my16)
            g16 = sp.tile([K, K], F16, tag="g16", name="g16")
            nc.vector.tensor_copy(g16, g_p)

            # resolvent scalars: ev = [ex | ey]; t = ev/max(ev); im = 1/(1+t)
            # re = sqrt(t)*im; both scaled by sqrt(LMBDA)
            evmax = sp.tile([1, 1], F32)
            nc.vector.tensor_reduce(evmax, ev_t, mybir.AxisListType.X,
                                    mybir.AluOpType.max)
            evrec = sp.tile([1, 1], F32)
            nc.vector.reciprocal(evrec, evmax)
            t_t = sp.tile([1, 2 * K], F32)
            nc.vector.tensor_scalar_mul(t_t, ev_t, evrec)
            tp1 = sp.tile([1, 2 * K], F32)
            nc.vector.tensor_scalar_add(tp1, t_t, 1.0)
            im_t = sp.tile([1, 2 * K], F32)
            nc.vector.reciprocal(im_t, tp1)
            sq_t = sp.tile([1, 2 * K], F32)
            nc.scalar.sqrt(sq_t, t_t)
            re_t = sp.tile([1, 2 * K], F32)
            nc.vector.tensor_mul(re_t, sq_t, im_t)
            nc.vector.tensor_scalar_mul(re_t, re_t, SQRT_LMBDA)
            nc.vector.tensor_scalar_mul(im_t, im_t, SQRT_LMBDA)

            # Newton-Schulz inverse (S symmetric PD), two-hop steps:
            #   B = 2I - S X  (DVE STT, fp16 out);  X' = X B  (X symmetric).
            # interleave(j) fills the PE-queue gaps with projection matmuls.
            def newton_inverse(mat_p, s16, tag, steps, interleave=None):
                rs = sp.tile([K, 1], F32, tag=f"{tag}_rs", name=f"{tag}_rs")
                nc.vector.tensor_reduce(rs, mat_p, mybir.AxisListType.X,
                                        mybir.AluOpType.add,
                                        apply_absolute_value=True)
                nc.gpsimd.partition_all_reduce(rs, rs, K, ReduceOp.max)
                al = sp.tile([K, 1], F32, tag=f"{tag}_al", name=f"{tag}_al")
                nc.vector.reciprocal(al, rs)
                x_s = sp.tile([K, K], F16, tag=f"{tag}_x0", name=f"{tag}_x0")
                nc.vector.tensor_scalar_mul(x_s, id64, al)
                it_i = 0
                for it in range(steps):
                    t1 = ps_tile([K, K])
                    nc.tensor.matmul(t1, s16, x_s)            # S X (S sym)
                    if interleave is not None:
                        interleave(it_i); it_i += 1
                    b16 = wp.tile([K, K], F16, tag=f"{tag}_b16",
                                  name=f"{tag}_b16")
                    nc.vector.scalar_tensor_tensor(
                        b16, id64, 2.0, t1,
                        op0=mybir.AluOpType.mult,
                        op1=mybir.AluOpType.subtract)         # 2I - S X
                    xn = ps_tile([K, K])
                    nc.tensor.matmul(xn, x_s, b16)            # X (2I - S X)
                    if interleave is not None:
                        interleave(it_i); it_i += 1
                    x_new = sp.tile([K, K], F16, tag=f"{tag}_x{it + 1}",
                                    name=f"{tag}_x{it + 1}")
                    if it == steps - 1:
                        nc.scalar.copy(x_new, xn)
                    else:
                        nc.vector.tensor_copy(x_new, xn)
                    x_s = x_new
                if interleave is not None:
                    interleave(1000)   # flush any remainder
                return x_s

            # ---- x projections interleaved into Newton-G's PE gaps --------
            with tc.tile_pool(name="pacc", bufs=1, space="PSUM") as pacc:
                at_p = pacc.tile([C, K], F32)    # A^T  [C,K]
                byt_p = pacc.tile([C, K], F32)   # By^T [C,K]

                xprog = {"n": 0}

                def emit_xproj(upto):
                    while xprog["n"] < min(upto, NB):
                        n = xprog["n"]
                        nc.tensor.matmul(at_p, fxh[:, n, :], pxh[:, n, :],
                                         start=(n == 0), stop=(n == NB - 1))
                        xprog["n"] += 1

                gi16 = newton_inverse(
                    g_p, g16, "gi", NEWTON_STEPS_G,
                    interleave=lambda j: emit_xproj(j * 6))
                emit_xproj(NB)

                at16 = sp.tile([C, K], F16, tag="at16", name="at16")
                nc.vector.tensor_copy(at16, at_p)

                # S~ = Mx^T (A A^T) Mx
                sa_p = ps_tile([K, K])
                nc.tensor.matmul(sa_p, at16, at16)          # A A^T
                sa16 = wp.tile([K, K], F16, tag="sa16", name="sa16")
                nc.vector.tensor_copy(sa16, sa_p)
                h1_p = ps_tile([K, K])
                nc.tensor.matmul(h1_p, sa16, mx16)          # S_A Mx (sym)
                h16 = wp.tile([K, K], F16, tag="h16", name="h16")
                nc.vector.tensor_copy(h16, h1_p)
                st_p = ps_tile([K, K])
                nc.tensor.matmul(st_p, mx16, h16)           # Mx^T S_A Mx
                st16 = sp.tile([K, K], F16, tag="st16", name="st16")
                nc.scalar.copy(st16, st_p)

                # ---- y projections interleaved into Newton-S's gaps -------
                yprog = {"n": 0}

                def emit_yproj(upto):
                    while yprog["n"] < min(upto, NB):
                        n = yprog["n"]
                        nc.tensor.matmul(byt_p, fyh[:, n, :], pyh[:, n, :],
                                         start=(n == 0), stop=(n == NB - 1))
                        yprog["n"] += 1

                si16 = newton_inverse(
                    st_p, st16, "si", NEWTON_STEPS_S,
                    interleave=lambda j: emit_yproj((j + 1) * 4))
                emit_yproj(NB)

                byt16 = sp.tile([C, K], F16, tag="byt16", name="byt16")
                nc.vector.tensor_copy(byt16, byt_p)

            # D1T[a,i] = re2[a] - re1[i]; D2T likewise from im (emitted late
            # so the tiny mask matmuls never stall the PE queue)
            d12t_s = sp.tile([K, 2 * K], F32)
            for idx, src in enumerate((re_t, im_t)):
                pa = ps_tile([K, K])
                nc.tensor.matmul(pa, src[0:1, K:2 * K], ones_row)  # v2[p]
                pb = ps_tile([K, K])
                nc.tensor.matmul(pb, ones_row, src[0:1, 0:K])      # v1[f]
                ta = wp.tile([K, K], F32, tag=f"dta{idx}", name=f"dta{idx}")
                nc.vector.tensor_copy(ta, pa)
                nc.vector.tensor_sub(d12t_s[:, idx * K:(idx + 1) * K], ta, pb)
            d12v = d12t_s[:, :].rearrange("p (a b) -> p a b", a=2)

            # ---- RHS' = G By A^T Mx (3 matmuls, G-symmetry trick) ---------
            byat_p = ps_tile([K, K])
            nc.tensor.matmul(byat_p, byt16, at16)       # By A^T
            byat16 = wp.tile([K, K], F16, tag="byat16", name="byat16")
            nc.scalar.copy(byat16, byat_p)
            s2_p = ps_tile([K, K])
            nc.tensor.matmul(s2_p, byat16, g16)         # (G ByA^T)^T (G sym)
            s2c = wp.tile([K, K], F16, tag="s2c", name="s2c")
            nc.scalar.copy(s2c, s2_p)
            rp_p = ps_tile([K, K])
            nc.tensor.matmul(rp_p, s2c, mx16)           # G ByA^T Mx

            # ------- PCG state ---------------------------------------------
            # rz_s = [r | z] fp32 (fused alpha-update); p16 fp16 direction;
            # q_s, s_s fp32; y accumulates Y^T in PSUM via matmul.
            rz_s = sp.tile([K, 2 * K], F32)
            w_s = sp.tile([K, K], F32)
            u16 = sp.tile([K, 2 * K], F16)
            p16 = sp.tile([K, K], F16)
            y_p = yp.tile([K, K], F32)
            r_sl = rz_s[:, 0:K]
            z_sl = rz_s[:, K:2 * K]
            qs_v = sp.tile([K, 2 * K], F32)   # [q | s] fused tile
            q_s = qs_v[:, 0:K]
            s_s = qs_v[:, K:2 * K]
            u16v = u16[:, :].rearrange("p (a b) -> p a b", a=2)
            z_bc = z_sl.rearrange("p (o b) -> p o b", o=1).broadcast_to(
                [K, 2, K])

            nc.vector.tensor_copy(r_sl, rp_p)
            r16 = wp.tile([K, K], F16, tag="x16", name="r16i")
            nc.scalar.copy(r16, rp_p)

            def precond_psum(x16, tag):
                """P^-1 x in PSUM via (Gi x)^T = mm(lhsT=x16, rhs=Gi)."""
                ut_p = ps_tile([K, K])
                nc.tensor.matmul(ut_p, x16, gi16)
                ut16 = wp.tile([K, K], F16, tag=f"{tag}_ut", name=f"{tag}_ut")
                nc.scalar.copy(ut16, ut_p)
                v_p = ps_tile([K, K])
                nc.tensor.matmul(v_p, ut16, si16)
                return v_p

            def matvec_z(z16, tag):
                """w = M z -> w_s; w16 returned for the precond."""
                nc.vector.tensor_mul(u16v, d12v, z_bc)   # [D1T*z | D2T*z]
                gzt_p = ps_tile([K, K])
                nc.tensor.matmul(gzt_p, z16, g16)         # (G z)^T
                gzt16 = wp.tile([K, K], F16, tag="mv_gzt", name="mv_gzt")
                nc.scalar.copy(gzt16, gzt_p)
                gu_p = ps_tile([K, 3 * K])
                nc.tensor.matmul(gu_p[:, 0:2 * K], g16, u16)   # G u (both)
                nc.tensor.matmul(gu_p[:, 2 * K:3 * K], gzt16, st16)  # (Gz)S~
                mm_s = wp.tile([K, 2 * K], F32, tag="mv_mm", name="mv_mm")
                nc.vector.tensor_mul(mm_s, d12t_s, gu_p[:, 0:2 * K])  # mask
                a1_s = wp.tile([K, K], F32, tag="mv_a1", name="mv_a1")
                nc.vector.tensor_add(a1_s, mm_s[:, 0:K], mm_s[:, K:2 * K])
                nc.vector.tensor_add(w_s, a1_s, gu_p[:, 2 * K:3 * K])
                w16 = wp.tile([K, K], F16, tag="x16", name=f"{tag}_w16")
                nc.scalar.copy(w16, w_s)
                return w16

            def dot_b(a_ap, b_ap, tag):
                """<a,b> broadcast to all partitions as [K,1] SBUF."""
                prod = wp.tile([K, K], F32, tag="dot_dm", name="dot_dm")
                acc = wp.tile([K, 1], F32, tag=f"{tag}_acc", name=f"{tag}_acc")
                nc.vector.scalar_tensor_tensor(
                    prod, a_ap, 1.0, b_ap,
                    op0=mybir.AluOpType.bypass, op1=mybir.AluOpType.mult,
                    accum_out=acc)
                nc.gpsimd.partition_all_reduce(acc, acc, K, ReduceOp.add)
                return acc

            # init: z = P^-1 r; p = z; w = Mz; v = P^-1 w; q = w, s = v
            z0_p = precond_psum(r16, "pcz")
            nc.vector.tensor_copy(z_sl, z0_p)
            nc.scalar.copy(p16, z0_p)
            z16 = wp.tile([K, K], F16, tag="z16", name="z16i")
            nc.scalar.copy(z16, z0_p)
            rz0 = dot_b(r_sl, z_sl, "rz")
            rzrec = wp.tile([K, 1], F32, tag="rzrec", name="rzrec")
            nc.vector.reciprocal(rzrec, rz0)
            rzneg = wp.tile([K, 1], F32, tag="rzneg", name="rzneg")
            nc.vector.tensor_scalar_mul(rzneg, rz0, -1.0)
            w16 = matvec_z(z16, "init")
            nc.vector.tensor_copy(q_s, w_s)
            v_p = precond_psum(w16, "pcv")
            nc.vector.tensor_copy(s_s, v_p)

            for it in range(N_ITERS):
                # ---- alpha = rz/<p,q>; fused [r|z] -= alpha [q|s] ----
                pq = dot_b(p16, q_s, "pq")
                pqr = wp.tile([K, 1], F32, tag="pqr", name="pqr")
                nc.vector.reciprocal(pqr, pq)
                def emit_y_update():
                    # y^T += alpha p^T (PSUM accumulate, off critical path;
                    # must be emitted BEFORE the p16 beta-update)
                    al = wp.tile([K, 1], F32, tag="al", name="al")
                    nc.scalar.mul(al, rz0, pqr)
                    ida = wp.tile([K, K], F16, tag="ida", name="ida")
                    nc.scalar.mul(ida, id16, al)          # alpha * I (fp16)
                    nc.tensor.matmul(y_p, p16, ida,
                                     start=(it == 0),
                                     stop=(it == N_ITERS - 1))

                if it < N_ITERS - 1:
                    an = wp.tile([K, 1], F32, tag="an", name="an")
                    nc.vector.tensor_mul(an, rzneg, pqr)
                    nc.vector.scalar_tensor_tensor(
                        rz_s, qs_v, an, rz_s,
                        op0=mybir.AluOpType.mult, op1=mybir.AluOpType.add)
                    z16 = wp.tile([K, K], F16, tag="z16", name=f"z16_{it}")
                    nc.scalar.copy(z16, z_sl)

                    # ---- rz_new; matvec + precond for next q,s ----
                    rz_new = dot_b(r_sl, z_sl, "rz")
                    w16 = matvec_z(z16, f"i{it}")
                    emit_y_update()
                    if it < N_ITERS - 2:
                        v_p = precond_psum(w16, "pcv")
                    bt = wp.tile([K, 1], F32, tag="bt", name="bt")
                    nc.vector.tensor_mul(bt, rz_new, rzrec)
                    nc.vector.scalar_tensor_tensor(
                        p16, p16, bt, z_sl,
                        op0=mybir.AluOpType.mult, op1=mybir.AluOpType.add)
                    nc.vector.scalar_tensor_tensor(
                        q_s, q_s, bt, w_s,
                        op0=mybir.AluOpType.mult, op1=mybir.AluOpType.add)
                    if it < N_ITERS - 2:
                        nc.vector.scalar_tensor_tensor(
                            s_s, s_s, bt, v_p,
                            op0=mybir.AluOpType.mult, op1=mybir.AluOpType.add)
                else:
                    emit_y_update()
                    break
                rz0 = rz_new
                rzrec = wp.tile([K, 1], F32, tag="rzrec", name="rzrec")
                nc.vector.reciprocal(rzrec, rz0)
                rzneg = wp.tile([K, 1], F32, tag="rzneg", name="rzneg")
                nc.vector.tensor_scalar_mul(rzneg, rz0, -1.0)

            # -------- output: C = Y Mx^T  (y_p holds Y^T) ------------------
            y16 = wp.tile([K, K], F16, tag="y16", name="y16")
            nc.scalar.copy(y16, y_p)
            c_p = ps_tile([K, K])
            nc.tensor.matmul(c_p, y16, mxT16)           # (Y^T)^T Mx^T
            c_s = wp.tile([K, K], F32, tag="c_s", name="c_s")
            nc.vector.tensor_copy(c_s, c_p)
            nc.sync.dma_start(out_d[:, :], c_s)

    nc.finalize()
    return nc


def get_program(shard: bool = False):
    if shard not in _PROGRAM_CACHE:
        _PROGRAM_CACHE[shard] = build_program(shard)
    return _PROGRAM_CACHE[shard]


def make_in_maps(inputs, shard: bool = False):
    fx = np.asarray(inputs["feat_x"], np.float32)[0].astype(np.float16)
    fy = np.asarray(inputs["feat_y"], np.float32)[0].astype(np.float16)
    pxT = np.ascontiguousarray(
        np.asarray(inputs["evecs_trans_x"], np.float32)[0].T).astype(np.float16)
    pyT = np.ascontiguousarray(
        np.asarray(inputs["evecs_trans_y"], np.float32)[0].T).astype(np.float16)
    mx = np.asarray(inputs["sqrtMk_x"], np.float32)[0]
    my = np.asarray(inputs["sqrtMk_y"], np.float32)[0]
    small = np.ascontiguousarray(np.concatenate([mx, my, mx.T], axis=1))
    ev = np.ascontiguousarray(np.concatenate([
        np.asarray(inputs["evals_x"], np.float32)[0],
        np.asarray(inputs["evals_y"], np.float32)[0],
    ])[None, :])
    m = {"fx": fx, "fy": fy, "pxT": pxT, "pyT": pyT,
         "small": small, "ev": ev}
    return [dict(m) for _ in range(N_CORES)]


def kernel(**inputs) -> np.ndarray:
    nc = get_program(SHARD)
    in_maps = make_in_maps(inputs, SHARD)
    res = run_bass_kernel_spmd(nc, in_maps, core_ids=list(range(N_CORES)))
    out = np.asarray(res.results[0]["out"], dtype=np.float32)
    return out[None]


# revision 13
# speedup vs baseline: 2.6257x; 1.0629x over previous
"""Trainium2 Bass kernel for nn_ExpandedResolventFMNet.

Mathematical reformulation (validated in fp64/fp16 against the jax reference):

The reference builds kron(A.T, My) [8192x4096], its Gram [4096^2], resolvent
kron masks, and solves a dense 4096x4096 system.  All of that collapses:

  first        = kron(A A^T, G),              G = My^T My
  second       = kron-sum of 64x64 factors; with X = Mx W the full system is
  M(W)         = S~ W G + LMBDA * sum_d Dd*( (Dd*W) G ) = R~    (* = Hadamard)
  S~           = Mx^T (A A^T) Mx
  R~           = G By A^T Mx,   By = Py fy
  Dd           = resolvent-mask difference matrices (64x64)
  output C     = (Mx W)^T

The device runs the transposed system Y = W^T:

  M'(Y) = G Y S~ + sum_d DdT * (G (DdT * Y)),   C = Y Mx^T

solved by PCG with the exact-kron preconditioner P^-1 x = Gi x Si, where
Gi, Si come from on-device Newton-Schulz iteration (two-hop steps:
X' = X (2I - S X)).  G's symmetry makes every matmul transpose-free, and
Y^T is accumulated in PSUM via matmul against an alpha-scaled identity so
the output needs no transpose.

Fully unsharded: every core redundantly computes the whole answer, so there
are no collectives (SPMD launch skew made the barrier + two AllReduce cost
~64us on the measured core).  All matmuls run in fp16 (single-pass, 1
cycle/row vs fp32's split LOW_HIGH 2-pass) with fp32 PSUM accumulation; CG
state stays fp32 except the search direction p (fp16).  Inputs are cast to
fp16 on the host (halves HBM traffic).  The four big tensors stream through
the gpsimd SWDGE queue, which stripes descriptors over all 16 SDMA engines
(the HWDGE rings only get 5); queue FIFO order gives the x-side strict
priority.  Each partition's data is contiguous in DRAM (125 descriptors
per tensor).
"""

import numpy as np

import concourse.bacc as bacc
import concourse.mybir as mybir
from concourse.bass_isa import ReduceOp
from concourse.bass_utils import run_bass_kernel_spmd
from concourse.masks import make_identity
from concourse.tile import TileContext

F32 = mybir.dt.float32
F16 = mybir.dt.float16
K = 64          # spectral basis size
C = 128         # feature channels
V = 5000        # vertices
P = 128         # DMA partition rows
NB = 39         # full contraction chunks (V = P * NB + TAIL)
TAIL = 8        # leftover rows handled by one small matmul
N_CORES = 8
N_ITERS = 6
NEWTON_STEPS_S = 6
NEWTON_STEPS_G = 4
SQRT_LMBDA = 10.0

SHARD = False   # kept for test.py compat; only the unsharded path exists

_PROGRAM_CACHE = {}


def build_program(shard: bool):
    nc = bacc.Bacc("TRN2", num_devices=N_CORES)

    CK = C + K      # concatenated row: [fx | pxT]
    xc_d = nc.dram_tensor("xc", [V, CK], F16, kind="ExternalInput")
    yc_d = nc.dram_tensor("yc", [V, CK], F16, kind="ExternalInput")
    # mx|my|mxT [64, 3*64] fp32 (host-concatenated)
    small_d = nc.dram_tensor("small", [K, 3 * K], F32, kind="ExternalInput")
    ev_d = nc.dram_tensor("ev", [1, 2 * K], F32, kind="ExternalInput")
    out_d = nc.dram_tensor("out", [K, K], F32, kind="ExternalOutput")

    xc_ap = xc_d[0:P * NB, :].rearrange("(p n) c -> p n c", p=P)
    yc_ap = yc_d[0:P * NB, :].rearrange("(p n) c -> p n c", p=P)

    with TileContext(nc) as tc:
        with (
            tc.tile_pool(name="big", bufs=1) as bp,
            tc.tile_pool(name="persist", bufs=1) as sp,
            tc.tile_pool(name="work", bufs=2) as wp,
            tc.tile_pool(name="psum", bufs=2, space="PSUM") as pp,
            tc.tile_pool(name="yacc", bufs=1, space="PSUM") as yp,
        ):
            _ps_state = {"i": 0}

            def ps_tile(shape):
                i = _ps_state["i"]
                _ps_state["i"] += 1
                return pp.tile(shape, F32, tag=f"ps{i % 2}", name=f"pst{i}")

            # ---------------- input DMA ------------------------------------
            # smalls ride the (otherwise idle) HWDGE queues; the four big
            # tensors stream through gpsimd SWDGE in x-first FIFO order.
            small_t = sp.tile([K, 3 * K], F32)
            ev_t = sp.tile([1, 2 * K], F32)
            xc_t = bp.tile([P, NB, CK], F16)
            yc_t = bp.tile([P, NB, CK], F16)
            xtl = sp.tile([TAIL, CK], F16)
            ytl = sp.tile([TAIL, CK], F16)
            H = NB // 2
            nc.sync.dma_start(small_t, small_d[:, :])
            nc.sync.dma_start(ev_t, ev_d[:, :])
            nc.sync.dma_start(xtl, xc_d[P * NB:V, :])
            nc.sync.dma_start(ytl, yc_d[P * NB:V, :])
            nc.gpsimd.dma_start(xc_t[:, 0:H, :], xc_ap[:, 0:H, :])
            nc.gpsimd.dma_start(xc_t[:, H:NB, :], xc_ap[:, H:NB, :])
            nc.gpsimd.dma_start(yc_t[:, 0:H, :], yc_ap[:, 0:H, :])
            nc.gpsimd.dma_start(yc_t[:, H:NB, :], yc_ap[:, H:NB, :])

            mx_s = small_t[:, 0:K]
            my_s = small_t[:, K:2 * K]
            mxT_s = small_t[:, 2 * K:3 * K]

            # fp16 copies of the small matrices (scalar engine)
            m16 = sp.tile([K, 3 * K], F16)
            nc.scalar.copy(m16[:, 0:K], mx_s)
            nc.scalar.copy(m16[:, K:2 * K], my_s)
            nc.scalar.copy(m16[:, 2 * K:3 * K], mxT_s)
            mx16 = m16[:, 0:K]
            my16 = m16[:, K:2 * K]
            mxT16 = m16[:, 2 * K:3 * K]

            ident = sp.tile([C, C], F32)
            make_identity(nc, ident)
            id64 = ident[0:K, 0:K]
            id16 = sp.tile([K, K], F16)
            nc.scalar.copy(id16, id64)
            ones_row = sp.tile([1, K], F32)
            nc.vector.memset(ones_row, 1.0)

            # ---------------- G = My^T My (early; data lands first) --------
            g_p = ps_tile([K, K])
            nc.tensor.matmul(g_p, my16, my16)
            g16 = sp.tile([K, K], F16, tag="g16", name="g16")
            nc.vector.tensor_copy(g16, g_p)

            # resolvent scalars: ev = [ex | ey]; t = ev/max(ev); im = 1/(1+t)
            # re = sqrt(t)*im; both scaled by sqrt(LMBDA)
            evmax = sp.tile([1, 1], F32)
            nc.vector.tensor_reduce(evmax, ev_t, mybir.AxisListType.X,
                                    mybir.AluOpType.max)
            evrec = sp.tile([1, 1], F32)
            nc.vector.reciprocal(evrec, evmax)
            t_t = sp.tile([1, 2 * K], F32)
            nc.vector.tensor_scalar_mul(t_t, ev_t, evrec)
            tp1 = sp.tile([1, 2 * K], F32)
            nc.vector.tensor_scalar_add(tp1, t_t, 1.0)
            im_t = sp.tile([1, 2 * K], F32)
            nc.vector.reciprocal(im_t, tp1)
            sq_t = sp.tile([1, 2 * K], F32)
            nc.scalar.sqrt(sq_t, t_t)
            re_t = sp.tile([1, 2 * K], F32)
            nc.vector.tensor_mul(re_t, sq_t, im_t)
            nc.vector.tensor_scalar_mul(re_t, re_t, SQRT_LMBDA)
            nc.vector.tensor_scalar_mul(im_t, im_t, SQRT_LMBDA)

            # Newton-Schulz inverse (S symmetric PD), two-hop steps:
            #   B = 2I - S X  (DVE STT, fp16 out);  X' = X B  (X symmetric).
            # interleave(j) fills the PE-queue gaps with projection matmuls.
            def newton_inverse(mat_p, s16, tag, steps, interleave=None):
                rs = sp.tile([K, 1], F32, tag=f"{tag}_rs", name=f"{tag}_rs")
                nc.vector.tensor_reduce(rs, mat_p, mybir.AxisListType.X,
                                        mybir.AluOpType.add,
                                        apply_absolute_value=True)
                nc.gpsimd.partition_all_reduce(rs, rs, K, ReduceOp.max)
                al = sp.tile([K, 1], F32, tag=f"{tag}_al", name=f"{tag}_al")
                nc.vector.reciprocal(al, rs)
                x_s = sp.tile([K, K], F16, tag=f"{tag}_x0", name=f"{tag}_x0")
                nc.vector.tensor_scalar_mul(x_s, id64, al)
                it_i = 0
                for it in range(steps):
                    t1 = ps_tile([K, K])
                    nc.tensor.matmul(t1, s16, x_s)            # S X (S sym)
                    if interleave is not None:
                        interleave(it_i); it_i += 1
                    b16 = wp.tile([K, K], F16, tag=f"{tag}_b16",
                                  name=f"{tag}_b16")
                    nc.vector.scalar_tensor_tensor(
                        b16, id64, 2.0, t1,
                        op0=mybir.AluOpType.mult,
                        op1=mybir.AluOpType.subtract)         # 2I - S X
                    xn = ps_tile([K, K])
                    nc.tensor.matmul(xn, x_s, b16)            # X (2I - S X)
                    if interleave is not None:
                        interleave(it_i); it_i += 1
                    x_new = sp.tile([K, K], F16, tag=f"{tag}_x{it + 1}",
                                    name=f"{tag}_x{it + 1}")
                    if it == steps - 1:
                        nc.scalar.copy(x_new, xn)
                    else:
                        nc.vector.tensor_copy(x_new, xn)
                    x_s = x_new
                if interleave is not None:
                    interleave(1000)   # flush any remainder
                return x_s

            # ---- x projections interleaved into Newton-G's PE gaps --------
            with tc.tile_pool(name="pacc", bufs=1, space="PSUM") as pacc:
                at_p = pacc.tile([C, K], F32)    # A^T  [C,K]
                byt_p = pacc.tile([C, K], F32)   # By^T [C,K]

                xprog = {"n": 0}

                def emit_xproj(upto):
                    while xprog["n"] < min(upto, NB + 1):
                        n = xprog["n"]
                        if n < NB:
                            nc.tensor.matmul(
                                at_p, xc_t[:, n, 0:C], xc_t[:, n, C:CK],
                                start=(n == 0), stop=False)
                        else:
                            nc.tensor.matmul(
                                at_p, xtl[:, 0:C], xtl[:, C:CK],
                                start=False, stop=True)
                        xprog["n"] += 1

                gi16 = newton_inverse(
                    g_p, g16, "gi", NEWTON_STEPS_G,
                    interleave=lambda j: emit_xproj(j * 6))
                emit_xproj(NB + 1)

                at16 = sp.tile([C, K], F16, tag="at16", name="at16")
                nc.vector.tensor_copy(at16, at_p)

                # S~ = Mx^T (A A^T) Mx
                sa_p = ps_tile([K, K])
                nc.tensor.matmul(sa_p, at16, at16)          # A A^T
                sa16 = wp.tile([K, K], F16, tag="sa16", name="sa16")
                nc.vector.tensor_copy(sa16, sa_p)
                h1_p = ps_tile([K, K])
                nc.tensor.matmul(h1_p, sa16, mx16)          # S_A Mx (sym)
                h16 = wp.tile([K, K], F16, tag="h16", name="h16")
                nc.vector.tensor_copy(h16, h1_p)
                st_p = ps_tile([K, K])
                nc.tensor.matmul(st_p, mx16, h16)           # Mx^T S_A Mx
                st16 = sp.tile([K, K], F16, tag="st16", name="st16")
                nc.scalar.copy(st16, st_p)

                # ---- y projections interleaved into Newton-S's gaps -------
                yprog = {"n": 0}

                def emit_yproj(upto):
                    while yprog["n"] < min(upto, NB + 1):
                        n = yprog["n"]
                        if n < NB:
                            nc.tensor.matmul(
                                byt_p, yc_t[:, n, 0:C], yc_t[:, n, C:CK],
                                start=(n == 0), stop=False)
                        else:
                            nc.tensor.matmul(
                                byt_p, ytl[:, 0:C], ytl[:, C:CK],
                                start=False, stop=True)
                        yprog["n"] += 1

                si16 = newton_inverse(
                    st_p, st16, "si", NEWTON_STEPS_S,
                    interleave=lambda j: emit_yproj((j + 1) * 4))
                emit_yproj(NB + 1)

                byt16 = sp.tile([C, K], F16, tag="byt16", name="byt16")
                nc.vector.tensor_copy(byt16, byt_p)

            # D1T[a,i] = re2[a] - re1[i]; D2T likewise from im (emitted late
            # so the tiny mask matmuls never stall the PE queue)
            d12t_s = sp.tile([K, 2 * K], F32)
            for idx, src in enumerate((re_t, im_t)):
                pa = ps_tile([K, K])
                nc.tensor.matmul(pa, src[0:1, K:2 * K], ones_row)  # v2[p]
                pb = ps_tile([K, K])
                nc.tensor.matmul(pb, ones_row, src[0:1, 0:K])      # v1[f]
                ta = wp.tile([K, K], F32, tag=f"dta{idx}", name=f"dta{idx}")
                nc.vector.tensor_copy(ta, pa)
                nc.vector.tensor_sub(d12t_s[:, idx * K:(idx + 1) * K], ta, pb)
            d12v = d12t_s[:, :].rearrange("p (a b) -> p a b", a=2)

            # ---- RHS' = G By A^T Mx (3 matmuls, G-symmetry trick) ---------
            byat_p = ps_tile([K, K])
            nc.tensor.matmul(byat_p, byt16, at16)       # By A^T
            byat16 = wp.tile([K, K], F16, tag="byat16", name="byat16")
            nc.scalar.copy(byat16, byat_p)
            s2_p = ps_tile([K, K])
            nc.tensor.matmul(s2_p, byat16, g16)         # (G ByA^T)^T (G sym)
            s2c = wp.tile([K, K], F16, tag="s2c", name="s2c")
            nc.scalar.copy(s2c, s2_p)
            rp_p = ps_tile([K, K])
            nc.tensor.matmul(rp_p, s2c, mx16)           # G ByA^T Mx

            # ------- PCG state ---------------------------------------------
            # rz_s = [r | z] fp32 (fused alpha-update); p16 fp16 direction;
            # q_s, s_s fp32; y accumulates Y^T in PSUM via matmul.
            rz_s = sp.tile([K, 2 * K], F32)
            w_s = sp.tile([K, K], F32)
            u16 = sp.tile([K, 2 * K], F16)
            p16 = sp.tile([K, K], F16)
            y_p = yp.tile([K, K], F32)
            r_sl = rz_s[:, 0:K]
            z_sl = rz_s[:, K:2 * K]
            qs_v = sp.tile([K, 2 * K], F32)   # [q | s] fused tile
            q_s = qs_v[:, 0:K]
            s_s = qs_v[:, K:2 * K]
            u16v = u16[:, :].rearrange("p (a b) -> p a b", a=2)
            z_bc = z_sl.rearrange("p (o b) -> p o b", o=1).broadcast_to(
                [K, 2, K])

            nc.vector.tensor_copy(r_sl, rp_p)
            r16 = wp.tile([K, K], F16, tag="x16", name="r16i")
            nc.scalar.copy(r16, rp_p)

            def precond_psum(x16, tag):
                """P^-1 x in PSUM via (Gi x)^T = mm(lhsT=x16, rhs=Gi)."""
                ut_p = ps_tile([K, K])
                nc.tensor.matmul(ut_p, x16, gi16)
                ut16 = wp.tile([K, K], F16, tag=f"{tag}_ut", name=f"{tag}_ut")
                nc.scalar.copy(ut16, ut_p)
                v_p = ps_tile([K, K])
                nc.tensor.matmul(v_p, ut16, si16)
                return v_p

            def matvec_z(z16, tag):
                """w = M z -> w_s; w16 returned for the precond."""
                nc.vector.tensor_mul(u16v, d12v, z_bc)   # [D1T*z | D2T*z]
                gzt_p = ps_tile([K, K])
                nc.tensor.matmul(gzt_p, z16, g16)         # (G z)^T
                gzt16 = wp.tile([K, K], F16, tag="mv_gzt", name="mv_gzt")
                nc.scalar.copy(gzt16, gzt_p)
                gu_p = ps_tile([K, 3 * K])
                nc.tensor.matmul(gu_p[:, 0:2 * K], g16, u16)   # G u (both)
                nc.tensor.matmul(gu_p[:, 2 * K:3 * K], gzt16, st16)  # (Gz)S~
                mm_s = wp.tile([K, 2 * K], F32, tag="mv_mm", name="mv_mm")
                nc.vector.tensor_mul(mm_s, d12t_s, gu_p[:, 0:2 * K])  # mask
                a1_s = wp.tile([K, K], F32, tag="mv_a1", name="mv_a1")
                nc.vector.tensor_add(a1_s, mm_s[:, 0:K], mm_s[:, K:2 * K])
                nc.vector.tensor_add(w_s, a1_s, gu_p[:, 2 * K:3 * K])
                w16 = wp.tile([K, K], F16, tag="x16", name=f"{tag}_w16")
                nc.scalar.copy(w16, w_s)
                return w16

            def dot_b(a_ap, b_ap, tag):
                """<a,b> broadcast to all partitions as [K,1] SBUF."""
                prod = wp.tile([K, K], F32, tag="dot_dm", name="dot_dm")
                acc = wp.tile([K, 1], F32, tag=f"{tag}_acc", name=f"{tag}_acc")
                nc.vector.scalar_tensor_tensor(
                    prod, a_ap, 1.0, b_ap,
                    op0=mybir.AluOpType.bypass, op1=mybir.AluOpType.mult,
                    accum_out=acc)
                nc.gpsimd.partition_all_reduce(acc, acc, K, ReduceOp.add)
                return acc

            # init: z = P^-1 r; p = z; w = Mz; v = P^-1 w; q = w, s = v
            z0_p = precond_psum(r16, "pcz")
            nc.vector.tensor_copy(z_sl, z0_p)
            nc.scalar.copy(p16, z0_p)
            z16 = wp.tile([K, K], F16, tag="z16", name="z16i")
            nc.scalar.copy(z16, z0_p)
            rz0 = dot_b(r_sl, z_sl, "rz")
            rzrec = wp.tile([K, 1], F32, tag="rzrec", name="rzrec")
            nc.vector.reciprocal(rzrec, rz0)
            rzneg = wp.tile([K, 1], F32, tag="rzneg", name="rzneg")
            nc.vector.tensor_scalar_mul(rzneg, rz0, -1.0)
            w16 = matvec_z(z16, "init")
            nc.vector.tensor_copy(q_s, w_s)
            v_p = precond_psum(w16, "pcv")
            nc.vector.tensor_copy(s_s, v_p)

            for it in range(N_ITERS):
                # ---- alpha = rz/<p,q>; fused [r|z] -= alpha [q|s] ----
                pq = dot_b(p16, q_s, "pq")
                pqr = wp.tile([K, 1], F32, tag="pqr", name="pqr")
                nc.vector.reciprocal(pqr, pq)
                def emit_y_update():
                    # y^T += alpha p^T (PSUM accumulate, off critical path;
                    # must be emitted BEFORE the p16 beta-update)
                    al = wp.tile([K, 1], F32, tag="al", name="al")
                    nc.scalar.mul(al, rz0, pqr)
                    ida = wp.tile([K, K], F16, tag="ida", name="ida")
                    nc.scalar.mul(ida, id16, al)          # alpha * I (fp16)
                    nc.tensor.matmul(y_p, p16, ida,
                                     start=(it == 0),
                                     stop=(it == N_ITERS - 1))

                if it < N_ITERS - 1:
                    an = wp.tile([K, 1], F32, tag="an", name="an")
                    nc.vector.tensor_mul(an, rzneg, pqr)
                    nc.vector.scalar_tensor_tensor(
                        rz_s, qs_v, an, rz_s,
                        op0=mybir.AluOpType.mult, op1=mybir.AluOpType.add)
                    z16 = wp.tile([K, K], F16, tag="z16", name=f"z16_{it}")
                    nc.scalar.copy(z16, z_sl)

                    # ---- rz_new; matvec + precond for next q,s ----
                    rz_new = dot_b(r_sl, z_sl, "rz")
                    w16 = matvec_z(z16, f"i{it}")
                    emit_y_update()
                    if it < N_ITERS - 2:
                        v_p = precond_psum(w16, "pcv")
                    bt = wp.tile([K, 1], F32, tag="bt", name="bt")
                    nc.vector.tensor_mul(bt, rz_new, rzrec)
                    nc.vector.scalar_tensor_tensor(
                        p16, p16, bt, z_sl,
                        op0=mybir.AluOpType.mult, op1=mybir.AluOpType.add)
                    nc.vector.scalar_tensor_tensor(
                        q_s, q_s, bt, w_s,
                        op0=mybir.AluOpType.mult, op1=mybir.AluOpType.add)
                    if it < N_ITERS - 2:
                        nc.vector.scalar_tensor_tensor(
                            s_s, s_s, bt, v_p,
                            op0=mybir.AluOpType.mult, op1=mybir.AluOpType.add)
                else:
                    emit_y_update()
                    break
                rz0 = rz_new
                rzrec = wp.tile([K, 1], F32, tag="rzrec", name="rzrec")
                nc.vector.reciprocal(rzrec, rz0)
                rzneg = wp.tile([K, 1], F32, tag="rzneg", name="rzneg")
                nc.vector.tensor_scalar_mul(rzneg, rz0, -1.0)

            # -------- output: C = Y Mx^T  (y_p holds Y^T) ------------------
            y16 = wp.tile([K, K], F16, tag="y16", name="y16")
            nc.scalar.copy(y16, y_p)
            c_p = ps_tile([K, K])
            nc.tensor.matmul(c_p, y16, mxT16)           # (Y^T)^T Mx^T
            c_s = wp.tile([K, K], F32, tag="c_s", name="c_s")
            nc.vector.tensor_copy(c_s, c_p)
            nc.sync.dma_start(out_d[:, :], c_s)

    nc.finalize()
    return nc


def get_program(shard: bool = False):
    if shard not in _PROGRAM_CACHE:
        _PROGRAM_CACHE[shard] = build_program(shard)
    return _PROGRAM_CACHE[shard]


def make_in_maps(inputs, shard: bool = False):
    fx = np.asarray(inputs["feat_x"], np.float32)[0]
    fy = np.asarray(inputs["feat_y"], np.float32)[0]
    pxT = np.asarray(inputs["evecs_trans_x"], np.float32)[0].T
    pyT = np.asarray(inputs["evecs_trans_y"], np.float32)[0].T
    xc = np.ascontiguousarray(
        np.concatenate([fx, pxT], axis=1)).astype(np.float16)
    yc = np.ascontiguousarray(
        np.concatenate([fy, pyT], axis=1)).astype(np.float16)
    mx = np.asarray(inputs["sqrtMk_x"], np.float32)[0]
    my = np.asarray(inputs["sqrtMk_y"], np.float32)[0]
    small = np.ascontiguousarray(np.concatenate([mx, my, mx.T], axis=1))
    ev = np.ascontiguousarray(np.concatenate([
        np.asarray(inputs["evals_x"], np.float32)[0],
        np.asarray(inputs["evals_y"], np.float32)[0],
    ])[None, :])
    m = {"xc": xc, "yc": yc, "small": small, "ev": ev}
    return [dict(m) for _ in range(N_CORES)]


def kernel(**inputs) -> np.ndarray:
    nc = get_program(SHARD)
    in_maps = make_in_maps(inputs, SHARD)
    res = run_bass_kernel_spmd(nc, in_maps, core_ids=list(range(N_CORES)))
    out = np.asarray(res.results[0]["out"], dtype=np.float32)
    return out[None]


# revision 14
# speedup vs baseline: 2.6711x; 1.0173x over previous
"""Trainium2 Bass kernel for nn_ExpandedResolventFMNet.

Mathematical reformulation (validated in fp64/fp16 against the jax reference):

The reference builds kron(A.T, My) [8192x4096], its Gram [4096^2], resolvent
kron masks, and solves a dense 4096x4096 system.  All of that collapses:

  first        = kron(A A^T, G),              G = My^T My
  second       = kron-sum of 64x64 factors; with X = Mx W the full system is
  M(W)         = S~ W G + LMBDA * sum_d Dd*( (Dd*W) G ) = R~    (* = Hadamard)
  S~           = Mx^T (A A^T) Mx
  R~           = G By A^T Mx,   By = Py fy
  Dd           = resolvent-mask difference matrices (64x64)
  output C     = (Mx W)^T

The device runs the transposed system Y = W^T:

  M'(Y) = G Y S~ + sum_d DdT * (G (DdT * Y)),   C = Y Mx^T

solved by PCG with the exact-kron preconditioner P^-1 x = Gi x Si, where
Gi, Si come from on-device Newton-Schulz iteration (two-hop steps:
X' = X (2I - S X)).  G's symmetry makes every matmul transpose-free, and
Y^T is accumulated in PSUM via matmul against an alpha-scaled identity so
the output needs no transpose.

Fully unsharded: every core redundantly computes the whole answer, so there
are no collectives (SPMD launch skew made the barrier + two AllReduce cost
~64us on the measured core).  All matmuls run in fp16 (single-pass, 1
cycle/row vs fp32's split LOW_HIGH 2-pass) with fp32 PSUM accumulation; CG
state stays fp32 except the search direction p (fp16).  Inputs are cast to
fp16 on the host (halves HBM traffic).  The four big tensors stream through
the gpsimd SWDGE queue, which stripes descriptors over all 16 SDMA engines
(the HWDGE rings only get 5); queue FIFO order gives the x-side strict
priority.  Each partition's data is contiguous in DRAM (125 descriptors
per tensor).
"""

import numpy as np

import concourse.bacc as bacc
import concourse.mybir as mybir
from concourse.bass_isa import ReduceOp
from concourse.bass_utils import run_bass_kernel_spmd
from concourse.masks import make_identity
from concourse.tile import TileContext

F32 = mybir.dt.float32
F16 = mybir.dt.float16
K = 64          # spectral basis size
C = 128         # feature channels
V = 5000        # vertices
P = 128         # DMA partition rows
NB = 39         # full contraction chunks (V = P * NB + TAIL)
TAIL = 8        # leftover rows handled by one small matmul
N_CORES = 8
N_ITERS = 6
NEWTON_STEPS_S = 6
NEWTON_STEPS_G = 4
SQRT_LMBDA = 10.0

SHARD = False   # kept for test.py compat; only the unsharded path exists

_PROGRAM_CACHE = {}


def build_program(shard: bool):
    nc = bacc.Bacc("TRN2", num_devices=N_CORES)

    CK = C + K      # concatenated row: [fx | pxT]
    xc_d = nc.dram_tensor("xc", [V, CK], F16, kind="ExternalInput")
    yc_d = nc.dram_tensor("yc", [V, CK], F16, kind="ExternalInput")
    # mx|my|mxT [64, 3*64] fp32 (host-concatenated)
    small_d = nc.dram_tensor("small", [K, 3 * K], F32, kind="ExternalInput")
    ev_d = nc.dram_tensor("ev", [1, 2 * K], F32, kind="ExternalInput")
    out_d = nc.dram_tensor("out", [K, K], F32, kind="ExternalOutput")

    xc_ap = xc_d[0:P * NB, :].rearrange("(p n) c -> p n c", p=P)
    yc_ap = yc_d[0:P * NB, :].rearrange("(p n) c -> p n c", p=P)

    with TileContext(nc) as tc:
        with (
            tc.tile_pool(name="big", bufs=1) as bp,
            tc.tile_pool(name="persist", bufs=1) as sp,
            tc.tile_pool(name="work", bufs=2) as wp,
            tc.tile_pool(name="psum", bufs=2, space="PSUM") as pp,
            tc.tile_pool(name="yacc", bufs=1, space="PSUM") as yp,
        ):
            _ps_state = {"i": 0}

            def ps_tile(shape):
                i = _ps_state["i"]
                _ps_state["i"] += 1
                return pp.tile(shape, F32, tag=f"ps{i % 2}", name=f"pst{i}")

            # ---------------- input DMA ------------------------------------
            # smalls ride the (otherwise idle) HWDGE queues; the four big
            # tensors stream through gpsimd SWDGE in x-first FIFO order.
            small_t = sp.tile([K, 3 * K], F32)
            ev_t = sp.tile([1, 2 * K], F32)
            xc_t = bp.tile([P, NB, CK], F16)
            yc_t = bp.tile([P, NB, CK], F16)
            xtl = sp.tile([TAIL, CK], F16)
            ytl = sp.tile([TAIL, CK], F16)
            H = NB // 2
            nc.sync.dma_start(small_t, small_d[:, :])
            nc.sync.dma_start(ev_t, ev_d[:, :])
            nc.sync.dma_start(xtl, xc_d[P * NB:V, :])
            nc.sync.dma_start(ytl, yc_d[P * NB:V, :])
            nc.gpsimd.dma_start(xc_t[:, 0:H, :], xc_ap[:, 0:H, :])
            nc.gpsimd.dma_start(xc_t[:, H:NB, :], xc_ap[:, H:NB, :])
            nc.gpsimd.dma_start(yc_t[:, 0:H, :], yc_ap[:, 0:H, :])
            nc.gpsimd.dma_start(yc_t[:, H:NB, :], yc_ap[:, H:NB, :])

            mx_s = small_t[:, 0:K]
            my_s = small_t[:, K:2 * K]
            mxT_s = small_t[:, 2 * K:3 * K]

            # fp16 copies of the small matrices (scalar engine)
            m16 = sp.tile([K, 3 * K], F16)
            nc.scalar.copy(m16[:, 0:K], mx_s)
            nc.scalar.copy(m16[:, K:2 * K], my_s)
            nc.scalar.copy(m16[:, 2 * K:3 * K], mxT_s)
            mx16 = m16[:, 0:K]
            my16 = m16[:, K:2 * K]
            mxT16 = m16[:, 2 * K:3 * K]

            ident = sp.tile([C, C], F32)
            make_identity(nc, ident)
            id64 = ident[0:K, 0:K]
            id16 = sp.tile([K, K], F16)
            nc.scalar.copy(id16, id64)
            ones_row = sp.tile([1, K], F32)
            nc.vector.memset(ones_row, 1.0)

            # ---------------- G = My^T My (early; data lands first) --------
            g_p = ps_tile([K, K])
            nc.tensor.matmul(g_p, my16, my16)
            g16 = sp.tile([K, K], F16, tag="g16", name="g16")
            nc.vector.tensor_copy(g16, g_p)

            # resolvent scalars: ev = [ex | ey]; t = ev/max(ev); im = 1/(1+t)
            # re = sqrt(t)*im; both scaled by sqrt(LMBDA)
            evmax = sp.tile([1, 1], F32)
            nc.vector.tensor_reduce(evmax, ev_t, mybir.AxisListType.X,
                                    mybir.AluOpType.max)
            evrec = sp.tile([1, 1], F32)
            nc.vector.reciprocal(evrec, evmax)
            t_t = sp.tile([1, 2 * K], F32)
            nc.vector.tensor_scalar_mul(t_t, ev_t, evrec)
            tp1 = sp.tile([1, 2 * K], F32)
            nc.vector.tensor_scalar_add(tp1, t_t, 1.0)
            im_t = sp.tile([1, 2 * K], F32)
            nc.vector.reciprocal(im_t, tp1)
            sq_t = sp.tile([1, 2 * K], F32)
            nc.scalar.sqrt(sq_t, t_t)
            re_t = sp.tile([1, 2 * K], F32)
            nc.vector.tensor_mul(re_t, sq_t, im_t)
            nc.vector.tensor_scalar_mul(re_t, re_t, SQRT_LMBDA)
            nc.vector.tensor_scalar_mul(im_t, im_t, SQRT_LMBDA)

            # Newton-Schulz inverse (S symmetric PD), two-hop steps:
            #   B = 2I - S X  (DVE STT, fp16 out);  X' = X B  (X symmetric).
            # interleave(j) fills the PE-queue gaps with projection matmuls.
            def newton_inverse(mat_p, s16, tag, steps, interleave=None):
                rs = sp.tile([K, 1], F32, tag=f"{tag}_rs", name=f"{tag}_rs")
                nc.vector.tensor_reduce(rs, mat_p, mybir.AxisListType.X,
                                        mybir.AluOpType.add,
                                        apply_absolute_value=True)
                nc.gpsimd.partition_all_reduce(rs, rs, K, ReduceOp.max)
                al = sp.tile([K, 1], F32, tag=f"{tag}_al", name=f"{tag}_al")
                nc.vector.reciprocal(al, rs)
                x_s = sp.tile([K, K], F16, tag=f"{tag}_x0", name=f"{tag}_x0")
                nc.vector.tensor_scalar_mul(x_s, id64, al)
                it_i = 0
                for it in range(steps):
                    t1 = ps_tile([K, K])
                    nc.tensor.matmul(t1, s16, x_s)            # S X (S sym)
                    if interleave is not None:
                        interleave(it_i); it_i += 1
                    b16 = wp.tile([K, K], F16, tag=f"{tag}_b16",
                                  name=f"{tag}_b16")
                    nc.vector.scalar_tensor_tensor(
                        b16, id64, 2.0, t1,
                        op0=mybir.AluOpType.mult,
                        op1=mybir.AluOpType.subtract)         # 2I - S X
                    xn = ps_tile([K, K])
                    nc.tensor.matmul(xn, x_s, b16)            # X (2I - S X)
                    if interleave is not None:
                        interleave(it_i); it_i += 1
                    x_new = sp.tile([K, K], F16, tag=f"{tag}_x{it + 1}",
                                    name=f"{tag}_x{it + 1}")
                    if it == steps - 1:
                        nc.scalar.copy(x_new, xn)
                    else:
                        nc.vector.tensor_copy(x_new, xn)
                    x_s = x_new
                if interleave is not None:
                    interleave(1000)   # flush any remainder
                return x_s

            # ---- x projections interleaved into Newton-G's PE gaps --------
            with tc.tile_pool(name="pacc", bufs=1, space="PSUM") as pacc:
                at_p = pacc.tile([C, K], F32)    # A^T  [C,K]
                byt_p = pacc.tile([C, K], F32)   # By^T [C,K]

                xprog = {"n": 0}

                def emit_xproj(upto):
                    while xprog["n"] < min(upto, NB + 1):
                        n = xprog["n"]
                        if n < NB:
                            nc.tensor.matmul(
                                at_p, xc_t[:, n, 0:C], xc_t[:, n, C:CK],
                                start=(n == 0), stop=False)
                        else:
                            nc.tensor.matmul(
                                at_p, xtl[:, 0:C], xtl[:, C:CK],
                                start=False, stop=True)
                        xprog["n"] += 1

                gi16 = newton_inverse(
                    g_p, g16, "gi", NEWTON_STEPS_G,
                    interleave=lambda j: emit_xproj(j * 6))
                emit_xproj(NB + 1)

                at16 = sp.tile([C, K], F16, tag="at16", name="at16")
                nc.vector.tensor_copy(at16, at_p)

                # S~ = Mx^T (A A^T) Mx
                sa_p = ps_tile([K, K])
                nc.tensor.matmul(sa_p, at16, at16)          # A A^T
                sa16 = wp.tile([K, K], F16, tag="sa16", name="sa16")
                nc.vector.tensor_copy(sa16, sa_p)
                h1_p = ps_tile([K, K])
                nc.tensor.matmul(h1_p, sa16, mx16)          # S_A Mx (sym)
                h16 = wp.tile([K, K], F16, tag="h16", name="h16")
                nc.vector.tensor_copy(h16, h1_p)
                st_p = ps_tile([K, K])
                nc.tensor.matmul(st_p, mx16, h16)           # Mx^T S_A Mx
                st16 = sp.tile([K, K], F16, tag="st16", name="st16")
                nc.scalar.copy(st16, st_p)

                # ---- y projections interleaved into Newton-S's gaps -------
                yprog = {"n": 0}

                def emit_yproj(upto):
                    while yprog["n"] < min(upto, NB + 1):
                        n = yprog["n"]
                        if n < NB:
                            nc.tensor.matmul(
                                byt_p, yc_t[:, n, 0:C], yc_t[:, n, C:CK],
                                start=(n == 0), stop=False)
                        else:
                            nc.tensor.matmul(
                                byt_p, ytl[:, 0:C], ytl[:, C:CK],
                                start=False, stop=True)
                        yprog["n"] += 1

                si16 = newton_inverse(
                    st_p, st16, "si", NEWTON_STEPS_S,
                    interleave=lambda j: emit_yproj((j - 3) * 6))
                emit_yproj(NB + 1)

                byt16 = sp.tile([C, K], F16, tag="byt16", name="byt16")
                nc.vector.tensor_copy(byt16, byt_p)

            # D1T[a,i] = re2[a] - re1[i]; D2T likewise from im (emitted late
            # so the tiny mask matmuls never stall the PE queue)
            d12t_s = sp.tile([K, 2 * K], F32)
            for idx, src in enumerate((re_t, im_t)):
                pa = ps_tile([K, K])
                nc.tensor.matmul(pa, src[0:1, K:2 * K], ones_row)  # v2[p]
                pb = ps_tile([K, K])
                nc.tensor.matmul(pb, ones_row, src[0:1, 0:K])      # v1[f]
                ta = wp.tile([K, K], F32, tag=f"dta{idx}", name=f"dta{idx}")
                nc.vector.tensor_copy(ta, pa)
                nc.vector.tensor_sub(d12t_s[:, idx * K:(idx + 1) * K], ta, pb)
            d12v = d12t_s[:, :].rearrange("p (a b) -> p a b", a=2)

            # ---- RHS' = G By A^T Mx (3 matmuls, G-symmetry trick) ---------
            byat_p = ps_tile([K, K])
            nc.tensor.matmul(byat_p, byt16, at16)       # By A^T
            byat16 = wp.tile([K, K], F16, tag="byat16", name="byat16")
            nc.scalar.copy(byat16, byat_p)
            s2_p = ps_tile([K, K])
            nc.tensor.matmul(s2_p, byat16, g16)         # (G ByA^T)^T (G sym)
            s2c = wp.tile([K, K], F16, tag="s2c", name="s2c")
            nc.scalar.copy(s2c, s2_p)
            rp_p = ps_tile([K, K])
            nc.tensor.matmul(rp_p, s2c, mx16)           # G ByA^T Mx

            # ------- PCG state ---------------------------------------------
            # rz_s = [r | z] fp32 (fused alpha-update); p16 fp16 direction;
            # q_s, s_s fp32; y accumulates Y^T in PSUM via matmul.
            rz_s = sp.tile([K, 2 * K], F32)
            w_s = sp.tile([K, K], F32)
            u16 = sp.tile([K, 2 * K], F16)
            p16 = sp.tile([K, K], F16)
            y_p = yp.tile([K, K], F32)
            r_sl = rz_s[:, 0:K]
            z_sl = rz_s[:, K:2 * K]
            qs_v = sp.tile([K, 2 * K], F32)   # [q | s] fused tile
            q_s = qs_v[:, 0:K]
            s_s = qs_v[:, K:2 * K]
            u16v = u16[:, :].rearrange("p (a b) -> p a b", a=2)
            z_bc = z_sl.rearrange("p (o b) -> p o b", o=1).broadcast_to(
                [K, 2, K])

            nc.vector.tensor_copy(r_sl, rp_p)
            r16 = wp.tile([K, K], F16, tag="x16", name="r16i")
            nc.scalar.copy(r16, rp_p)

            def precond_psum(x16, tag):
                """P^-1 x in PSUM via (Gi x)^T = mm(lhsT=x16, rhs=Gi)."""
                ut_p = ps_tile([K, K])
                nc.tensor.matmul(ut_p, x16, gi16)
                ut16 = wp.tile([K, K], F16, tag=f"{tag}_ut", name=f"{tag}_ut")
                nc.scalar.copy(ut16, ut_p)
                v_p = ps_tile([K, K])
                nc.tensor.matmul(v_p, ut16, si16)
                return v_p

            def matvec_z(z16, tag):
                """w = M z -> w_s; w16 returned for the precond."""
                nc.vector.tensor_mul(u16v, d12v, z_bc)   # [D1T*z | D2T*z]
                gzt_p = ps_tile([K, K])
                nc.tensor.matmul(gzt_p, z16, g16)         # (G z)^T
                gzt16 = wp.tile([K, K], F16, tag="mv_gzt", name="mv_gzt")
                nc.vector.tensor_copy(gzt16, gzt_p)
                gu_p = ps_tile([K, 3 * K])
                nc.tensor.matmul(gu_p[:, 0:2 * K], g16, u16)   # G u (both)
                nc.tensor.matmul(gu_p[:, 2 * K:3 * K], gzt16, st16)  # (Gz)S~
                mm_s = wp.tile([K, 2 * K], F32, tag="mv_mm", name="mv_mm")
                nc.vector.tensor_mul(mm_s, d12t_s, gu_p[:, 0:2 * K])  # mask
                a1_s = wp.tile([K, K], F32, tag="mv_a1", name="mv_a1")
                nc.vector.tensor_add(a1_s, mm_s[:, 0:K], mm_s[:, K:2 * K])
                nc.vector.tensor_add(w_s, a1_s, gu_p[:, 2 * K:3 * K])
                w16 = wp.tile([K, K], F16, tag="x16", name=f"{tag}_w16")
                nc.scalar.copy(w16, w_s)
                return w16

            def dot_b(a_ap, b_ap, tag):
                """<a,b> broadcast to all partitions as [K,1] SBUF."""
                prod = wp.tile([K, K], F32, tag="dot_dm", name="dot_dm")
                acc = wp.tile([K, 1], F32, tag=f"{tag}_acc", name=f"{tag}_acc")
                nc.vector.scalar_tensor_tensor(
                    prod, a_ap, 1.0, b_ap,
                    op0=mybir.AluOpType.bypass, op1=mybir.AluOpType.mult,
                    accum_out=acc)
                nc.gpsimd.partition_all_reduce(acc, acc, K, ReduceOp.add)
                return acc

            # init: z = P^-1 r; p = z; w = Mz; v = P^-1 w; q = w, s = v
            z0_p = precond_psum(r16, "pcz")
            nc.vector.tensor_copy(z_sl, z0_p)
            nc.scalar.copy(p16, z0_p)
            z16 = wp.tile([K, K], F16, tag="z16", name="z16i")
            nc.scalar.copy(z16, z0_p)
            rz0 = dot_b(r_sl, z_sl, "rz")
            rzrec = wp.tile([K, 1], F32, tag="rzrec", name="rzrec")
            nc.vector.reciprocal(rzrec, rz0)
            rzneg = wp.tile([K, 1], F32, tag="rzneg", name="rzneg")
            nc.vector.tensor_scalar_mul(rzneg, rz0, -1.0)
            w16 = matvec_z(z16, "init")
            nc.vector.tensor_copy(q_s, w_s)
            v_p = precond_psum(w16, "pcv")
            nc.vector.tensor_copy(s_s, v_p)

            for it in range(N_ITERS):
                # ---- alpha = rz/<p,q>; fused [r|z] -= alpha [q|s] ----
                pq = dot_b(p16, q_s, "pq")
                pqr = wp.tile([K, 1], F32, tag="pqr", name="pqr")
                nc.vector.reciprocal(pqr, pq)
                def emit_y_update():
                    # y^T += alpha p^T (PSUM accumulate, off critical path;
                    # must be emitted BEFORE the p16 beta-update)
                    al = wp.tile([K, 1], F32, tag="al", name="al")
                    nc.scalar.mul(al, rz0, pqr)
                    ida = wp.tile([K, K], F16, tag="ida", name="ida")
                    nc.scalar.mul(ida, id16, al)          # alpha * I (fp16)
                    nc.tensor.matmul(y_p, p16, ida,
                                     start=(it == 0),
                                     stop=(it == N_ITERS - 1))

                if it < N_ITERS - 1:
                    an = wp.tile([K, 1], F32, tag="an", name="an")
                    nc.vector.tensor_mul(an, rzneg, pqr)
                    nc.vector.scalar_tensor_tensor(
                        rz_s, qs_v, an, rz_s,
                        op0=mybir.AluOpType.mult, op1=mybir.AluOpType.add)
                    z16 = wp.tile([K, K], F16, tag="z16", name=f"z16_{it}")
                    nc.vector.tensor_copy(z16, z_sl)

                    # ---- rz_new; matvec + precond for next q,s ----
                    rz_new = dot_b(r_sl, z_sl, "rz")
                    w16 = matvec_z(z16, f"i{it}")
                    emit_y_update()
                    if it < N_ITERS - 2:
                        v_p = precond_psum(w16, "pcv")
                    bt = wp.tile([K, 1], F32, tag="bt", name="bt")
                    nc.vector.tensor_mul(bt, rz_new, rzrec)
                    nc.vector.scalar_tensor_tensor(
                        p16, p16, bt, z_sl,
                        op0=mybir.AluOpType.mult, op1=mybir.AluOpType.add)
                    nc.vector.scalar_tensor_tensor(
                        q_s, q_s, bt, w_s,
                        op0=mybir.AluOpType.mult, op1=mybir.AluOpType.add)
                    if it < N_ITERS - 2:
                        nc.vector.scalar_tensor_tensor(
                            s_s, s_s, bt, v_p,
                            op0=mybir.AluOpType.mult, op1=mybir.AluOpType.add)
                else:
                    emit_y_update()
                    break
                rz0 = rz_new
                rzrec = wp.tile([K, 1], F32, tag="rzrec", name="rzrec")
                nc.vector.reciprocal(rzrec, rz0)
                rzneg = wp.tile([K, 1], F32, tag="rzneg", name="rzneg")
                nc.vector.tensor_scalar_mul(rzneg, rz0, -1.0)

            # -------- output: C = Y Mx^T  (y_p holds Y^T) ------------------
            y16 = wp.tile([K, K], F16, tag="y16", name="y16")
            nc.scalar.copy(y16, y_p)
            c_p = ps_tile([K, K])
            nc.tensor.matmul(c_p, y16, mxT16)           # (Y^T)^T Mx^T
            c_s = wp.tile([K, K], F32, tag="c_s", name="c_s")
            nc.vector.tensor_copy(c_s, c_p)
            nc.sync.dma_start(out_d[:, :], c_s)

    nc.finalize()
    return nc


def get_program(shard: bool = False):
    if shard not in _PROGRAM_CACHE:
        _PROGRAM_CACHE[shard] = build_program(shard)
    return _PROGRAM_CACHE[shard]


def make_in_maps(inputs, shard: bool = False):
    fx = np.asarray(inputs["feat_x"], np.float32)[0]
    fy = np.asarray(inputs["feat_y"], np.float32)[0]
    pxT = np.asarray(inputs["evecs_trans_x"], np.float32)[0].T
    pyT = np.asarray(inputs["evecs_trans_y"], np.float32)[0].T
    xc = np.ascontiguousarray(
        np.concatenate([fx, pxT], axis=1)).astype(np.float16)
    yc = np.ascontiguousarray(
        np.concatenate([fy, pyT], axis=1)).astype(np.float16)
    mx = np.asarray(inputs["sqrtMk_x"], np.float32)[0]
    my = np.asarray(inputs["sqrtMk_y"], np.float32)[0]
    small = np.ascontiguousarray(np.concatenate([mx, my, mx.T], axis=1))
    ev = np.ascontiguousarray(np.concatenate([
        np.asarray(inputs["evals_x"], np.float32)[0],
        np.asarray(inputs["evals_y"], np.float32)[0],
    ])[None, :])
    m = {"xc": xc, "yc": yc, "small": small, "ev": ev}
    return [dict(m) for _ in range(N_CORES)]


def kernel(**inputs) -> np.ndarray:
    nc = get_program(SHARD)
    in_maps = make_in_maps(inputs, SHARD)
    res = run_bass_kernel_spmd(nc, in_maps, core_ids=list(range(N_CORES)))
    out = np.asarray(res.results[0]["out"], dtype=np.float32)
    return out[None]
